# revision 8
# baseline (speedup 1.0000x reference)
"""BBox+Cls decoder TRN2 kernel (data-parallel over batch, 8 NeuronCores).

Per core: one batch element b -> 6144 tokens x 512 features through two
3-layer MLP heads (bbox->4, cls->91), sigmoid, cxcywh->xyxy transform and a
closed-form track-ID assignment (the reference's sequential scan reduces to
cumulative sums because IDs, once assigned, never change).

Matmuls run as float32r (1 PE cycle/row at N>=256 vs 4 for fp32; ~1e-4 rel
error measured on HW). Activations stay feature-major ([D, tokens]) so each
layer feeds the next without transposes; only the input is transposed, via
the PE. The final layer uses the feature-major activations as the stationary
operand, which lands token-major output ready for contiguous DMA.
"""

import numpy as np

import concourse.bass as bass
import concourse.mybir as mybir
import concourse.tile as tile
from concourse import bass_utils

F32 = mybir.dt.float32
F32R = mybir.dt.float32r
I32 = mybir.dt.int32
ALU = mybir.AluOpType
ACTF = mybir.ActivationFunctionType
AX = mybir.AxisListType

B, T, Q, D, H, C = 8, 24, 256, 512, 512, 91
IMG_W, IMG_H = 1088.0, 608.0
P = 128
NTOK = T * Q            # 6144 tokens per core
NTILE = 512             # tokens per n-tile
NT = NTOK // NTILE      # 12 n-tiles
GPT = NTILE // P        # 4 token groups (of 128) per n-tile
G = NTOK // P           # 48 token groups per core
KO = D // P             # 4 contraction chunks

TRACE = False
LAST_EXEC_TIME_NS = None


def _split_multiwait(nc):
    # This walrus build rejects >1 sync-wait per engine instruction; hoist
    # extras onto preceding same-engine NoOps (sequencer executes waits in
    # program order, so semantics are identical).
    for f in nc.m.functions:
        for blk in f.blocks:
            out, changed = [], False
            for ins in blk.instructions:
                si = ins.sync_info
                if si is not None and len(si.on_wait) > 1:
                    waits = list(si.on_wait)
                    for j, w in enumerate(waits[:-1]):
                        nd = mybir.InstNoOp(name=f"{ins.name}_wsplit{j}", ins=[], outs=[])
                        nd.engine = ins.engine
                        nd.sync_info = mybir.SyncInfo(on_wait=[w], on_update=[])
                        out.append(nd)
                    si.on_wait = waits[-1:]
                    ins.sync_info = si
                    changed = True
                out.append(ins)
            if changed:
                blk.instructions = out


def _build():
    nc = bass.Bass("TRN2", target_bir_lowering=False, debug=False, num_devices=8)

    feat = nc.dram_tensor("feat", [NTOK, D], F32, kind="ExternalInput").ap()
    wd, bd = {}, {}
    for head, fin in (("b", 4), ("c", C)):
        for li, (n, k) in enumerate([(D, H), (H, H), (H, fin)]):
            wd[head, li] = nc.dram_tensor(f"w{li}{head}", [n, k], F32, kind="ExternalInput").ap()
            bd[head, li] = nc.dram_tensor(f"b{li}{head}", [k], F32, kind="ExternalInput").ap()
    ident_d = nc.dram_tensor("ident", [P, P], F32, kind="ExternalInput").ap()
    tri_i_d = nc.dram_tensor("tri_incl", [T, T], F32, kind="ExternalInput").ap()
    tri_e_d = nc.dram_tensor("tri_excl", [T, T], F32, kind="ExternalInput").ap()
    ones24_d = nc.dram_tensor("ones24", [T, T], F32, kind="ExternalInput").ap()
    ones1_d = nc.dram_tensor("ones1", [1, P], F32, kind="ExternalInput").ap()

    o_bbox = nc.dram_tensor("o_bbox", [NTOK, 4], F32, kind="ExternalOutput").ap()
    o_cls = nc.dram_tensor("o_cls", [NTOK, C], F32, kind="ExternalOutput").ap()
    o_ori = nc.dram_tensor("o_ori", [NTOK, 4], F32, kind="ExternalOutput").ap()
    o_ids = nc.dram_tensor("o_ids", [T, Q], I32, kind="ExternalOutput").ap()

    with tile.TileContext(nc) as tc:
        with (
            tc.tile_pool(name="const", bufs=1) as cpool,
            tc.tile_pool(name="io", bufs=2) as iopool,
            tc.tile_pool(name="act", bufs=2) as apool,
            tc.tile_pool(name="big", bufs=1) as bigpool,
            tc.tile_pool(name="idp", bufs=1) as idpool,
            tc.tile_pool(name="psT", bufs=2, space="PSUM") as psT,
            tc.tile_pool(name="psH", bufs=2, space="PSUM") as psH,
            tc.tile_pool(name="psF", bufs=1, space="PSUM") as psF,
            tc.tile_pool(name="psI", bufs=1, space="PSUM") as psI,
        ):
            # ---- constants: DMA fp32 then DVE-convert (to f32r where used
            # by f32r matmuls; plain copies otherwise so consumers sync on
            # the single DVE semaphore instead of scattered DMA lanes).
            def conv(dram_ap, shape, dt, name):
                raw = cpool.tile(shape, F32, tag=f"{name}_raw")
                nc.sync.dma_start(raw[:], dram_ap)
                t = cpool.tile(shape, dt, tag=name)
                nc.vector.tensor_copy(t[:], raw[:])
                return t

            w = {}
            for head in ("b", "c"):
                for li in range(2):
                    w[head, li] = conv(
                        wd[head, li].rearrange("(ko p) h -> p ko h", p=P),
                        [P, KO, H], F32R, f"w{li}{head}")
            # final layers run plain fp32 (fp32r has dst-pattern ISA
            # restrictions at small N; these matmuls are tiny anyway)
            w["b", 2] = conv(wd["b", 2].rearrange("(ko p) h -> p ko h", p=P),
                             [P, KO, 4], F32, "w2b")
            w["c", 2] = conv(wd["c", 2].rearrange("(ko p) h -> p ko h", p=P),
                             [P, KO, C], F32, "w2c")
            bcol = {}
            for head in ("b", "c"):
                for li in range(2):
                    bcol[head, li] = conv(
                        bd[head, li].rearrange("(m p) -> p m", p=P),
                        [P, KO], F32, f"b{li}{head}")
            b3row = {
                "b": conv(bd["b", 2][None, :], [1, 4], F32, "b3b"),
                "c": conv(bd["c", 2][None, :], [1, C], F32, "b3c"),
            }
            ident = conv(ident_d, [P, P], F32, "ident")
            tri_i = conv(tri_i_d, [T, T], F32, "tri_i")
            tri_e = conv(tri_e_d, [T, T], F32, "tri_e")
            ones24 = conv(ones24_d, [T, T], F32, "ones24")
            ones1 = conv(ones1_d, [1, P], F32, "ones1")

            # ---- persistent output staging (single-use; avoids WAR waits
            # on DMA completion lanes)
            clsbuf = bigpool.tile([P, G, C], F32)        # cls logits -> sigmoid in place
            bb_all = bigpool.tile([P, G, 4], F32)        # bbox logits -> sigmoid in place
            ori_all = bigpool.tile([P, G, 4], F32)
            ml = [bigpool.tile([P, T], F32, tag=f"ml{h}", name=f"ml{h}")
                  for h in range(2)]

            feat_r = feat.rearrange("(nt g p) d -> nt p g d", nt=NT, p=P)

            x_tm = [None] * NT
            xT = [None] * NT

            def load_xtm(nt):
                t = iopool.tile([P, GPT, D], F32, tag="x_tm")
                nc.sync.dma_start(t[:], feat_r[nt])
                x_tm[nt] = t

            def transpose_in(nt):
                # x_tm [tok(P), g, d] -> xT [d(P), ko, tok] via PE transpose
                xt = apool.tile([P, KO, NTILE], F32R, tag="xT")
                for g in range(GPT):
                    pt = psT.tile([P, KO * P], F32)
                    for ko in range(KO):
                        nc.tensor.transpose(
                            pt[:, ko * P:(ko + 1) * P],
                            x_tm[nt][:, g, ko * P:(ko + 1) * P],
                            ident[:],
                        )
                    nc.vector.tensor_copy(
                        xT_view(xt, g),
                        pt[:].rearrange("p (ko q) -> p ko q", ko=KO),
                    )
                xT[nt] = xt

            def xT_view(xt, g):
                return xt[:, :, g * P:(g + 1) * P]

            def mlp_layer(rhs, wt, bias, out_tag, out_dt=F32R):
                # rhs [P, KO, NTILE] f32r; out h^T [P, KO(m), NTILE]
                out = apool.tile([P, KO, NTILE], out_dt, tag=out_tag,
                                 name=out_tag)
                for m in range(KO):
                    pt = psH.tile([P, NTILE], F32)
                    for ko in range(KO):
                        nc.tensor.matmul(
                            pt[:], wt[:, ko, m * P:(m + 1) * P], rhs[:, ko, :],
                            start=(ko == 0), stop=(ko == KO - 1))
                    if m < 2:
                        nc.vector.tensor_scalar(
                            out[:, m, :], pt[:], bias[:, m:m + 1], 0.0,
                            ALU.add, ALU.max)
                    else:
                        nc.scalar.activation(
                            out[:, m, :], pt[:], ACTF.Relu,
                            bias=bias[:, m:m + 1], scale=1.0)
                return out

            def final_layer(nt, h2, head):
                fin = 4 if head == "b" else C
                for g in range(GPT):
                    j = nt * GPT + g
                    pt = psF.tile([P, fin], F32, tag=f"psF{head}")
                    for ko in range(KO):
                        nc.tensor.matmul(
                            pt[:], h2[:, ko, g * P:(g + 1) * P], w[head, 2][:, ko, :],
                            start=(ko == 0), stop=False)
                    nc.tensor.matmul(pt[:], ones1[:], b3row[head][:],
                                     start=False, stop=True)
                    if head == "b":
                        nc.vector.tensor_copy(bb_all[:, j, :], pt[:])
                    else:
                        nc.vector.tensor_copy(clsbuf[:, j, :], pt[:])
                        nc.vector.tensor_reduce(
                            ml[j % 2][:, j // 2:j // 2 + 1], clsbuf[:, j, :],
                            axis=AX.X, op=ALU.max)

            # ---- main loop
            load_xtm(0)
            transpose_in(0)
            for nt in range(NT):
                if nt + 1 < NT:
                    load_xtm(nt + 1)
                    transpose_in(nt + 1)
                for head in ("b", "c"):
                    h1 = mlp_layer(xT[nt], w[head, 0], bcol[head, 0], f"h1{head}")
                    h2 = mlp_layer(h1, w[head, 1], bcol[head, 1], f"h2{head}",
                                   out_dt=F32)
                    final_layer(nt, h2, head)
                x_tm[nt] = xT[nt] = None

            # ---- sigmoids (one big ACT op each, in place)
            nc.scalar.activation(
                clsbuf[:].rearrange("p g c -> p (g c)"),
                clsbuf[:].rearrange("p g c -> p (g c)"), ACTF.Sigmoid)
            nc.scalar.activation(
                bb_all[:].rearrange("p g c -> p (g c)"),
                bb_all[:].rearrange("p g c -> p (g c)"), ACTF.Sigmoid)

            # ---- ori: cxcywh -> xyxy scaled (columns of bb_all)
            cx, cy, ww, hh = (bb_all[:, :, i] for i in range(4))
            uw = idpool.tile([P, G], F32, tag="uw")
            uh = idpool.tile([P, G], F32, tag="uh")
            nc.vector.tensor_scalar(uw[:], ww, 0.5, None, ALU.mult)
            nc.vector.tensor_scalar(uh[:], hh, 0.5, None, ALU.mult)
            for col, a, u, s in ((0, cx, uw, IMG_W), (1, cy, uh, IMG_H),
                                 (2, cx, uw, IMG_W), (3, cy, uh, IMG_H)):
                op = ALU.subtract if col < 2 else ALU.add
                nc.vector.tensor_tensor(ori_all[:, :, col], a, u[:], op)
                nc.vector.tensor_scalar(ori_all[:, :, col], ori_all[:, :, col],
                                        s, None, ALU.mult)

            # ---- track IDs (closed form)
            # scores s24[t, q] = max cls logit; crossing iff logit >= 0
            s24 = idpool.tile([T, Q], F32, tag="s24")
            for h in range(2):
                pt = psI.tile([T, P], F32, tag="psS")
                nc.tensor.transpose(pt[:], ml[h][:], ident[:])
                nc.vector.tensor_copy(s24[:, h * P:(h + 1) * P], pt[:])
            m24 = idpool.tile([T, Q], F32, tag="m24")
            nc.vector.tensor_scalar(m24[:], s24[:], 0.0, None, ALU.is_ge)
            c24p = psI.tile([T, Q], F32, tag="psC")
            nc.tensor.matmul(c24p[:], tri_i[:], m24[:], start=True, stop=True)
            vmask = idpool.tile([T, Q], F32, tag="vmask")
            nc.vector.tensor_scalar(vmask[:], c24p[:], 0.5, None, ALU.is_ge)
            born = idpool.tile([T, Q], F32, tag="born")
            nc.vector.tensor_scalar(born[:], c24p[:], 1.0, None, ALU.is_equal)
            nc.vector.tensor_tensor(born[:], born[:], m24[:], ALU.mult)
            rowtot = idpool.tile([T, 1], F32, tag="rowtot")
            nc.vector.tensor_reduce(rowtot[:], born[:], axis=AX.X, op=ALU.add)
            exlp = psI.tile([T, Q], F32, tag="psC")
            nc.tensor.matmul(exlp[:, :1], tri_e[:], rowtot[:], start=True, stop=True)
            excl = idpool.tile([T, 1], F32, tag="excl")
            nc.vector.tensor_copy(excl[:], exlp[:, :1])
            # exclusive prefix over q (shift-add doubling), ping-pong tiles
            qp = [idpool.tile([T, Q], F32, tag=f"qp{i}", name=f"qp{i}")
                  for i in range(2)]
            nc.vector.memset(qp[0][:, :1], 0)
            nc.vector.tensor_copy(qp[0][:, 1:], born[:, :Q - 1])
            cur = 0
            for sh in (1, 2, 4, 8, 16, 32, 64, 128):
                src, dst = qp[cur], qp[1 - cur]
                nc.vector.tensor_copy(dst[:, :sh], src[:, :sh])
                nc.vector.tensor_tensor(dst[:, sh:], src[:, sh:], src[:, :Q - sh], ALU.add)
                cur = 1 - cur
            idval1 = qp[cur]
            nc.vector.tensor_scalar(idval1[:], idval1[:], excl[:], 1.0, ALU.add, ALU.add)
            pick = qp[1 - cur]
            nc.vector.tensor_tensor(pick[:], born[:], idval1[:], ALU.mult)
            idqp = psI.tile([T, Q], F32, tag="psC")
            nc.tensor.matmul(idqp[:], ones24[:], pick[:], start=True, stop=True)
            ids_f = idpool.tile([T, Q], F32, tag="ids_f")
            nc.vector.tensor_tensor(ids_f[:], vmask[:], idqp[:], ALU.mult)
            nc.vector.tensor_scalar(ids_f[:], ids_f[:], 1.0, None, ALU.subtract)
            ids_i = idpool.tile([T, Q], I32, tag="ids_i")
            nc.vector.tensor_copy(ids_i[:], ids_f[:])

            # ---- outputs
            nc.sync.dma_start(o_cls.rearrange("(g p) c -> p g c", p=P), clsbuf[:])
            nc.sync.dma_start(o_bbox.rearrange("(g p) c -> p g c", p=P), bb_all[:])
            nc.sync.dma_start(o_ori.rearrange("(g p) c -> p g c", p=P), ori_all[:])
            nc.sync.dma_start(o_ids, ids_i[:])

    _split_multiwait(nc)
    return nc


_NC = None


def kernel(object_features,
           bbox_w0, bbox_b0, bbox_w1, bbox_b1, bbox_w2, bbox_b2,
           cls_w0, cls_b0, cls_w1, cls_b1, cls_w2, cls_b2):
    global _NC, LAST_EXEC_TIME_NS
    if _NC is None:
        _NC = _build()
    nc = _NC

    consts = {
        "ident": np.eye(P, dtype=np.float32),
        "tri_incl": np.triu(np.ones((T, T), np.float32)),
        "tri_excl": np.triu(np.ones((T, T), np.float32), 1),
        "ones24": np.ones((T, T), np.float32),
        "ones1": np.ones((1, P), np.float32),
    }
    shared = {
        "w0b": bbox_w0, "w1b": bbox_w1, "w2b": bbox_w2,
        "b0b": bbox_b0, "b1b": bbox_b1, "b2b": bbox_b2,
        "w0c": cls_w0, "w1c": cls_w1, "w2c": cls_w2,
        "b0c": cls_b0, "b1c": cls_b1, "b2c": cls_b2,
    }
    shared = {k: np.ascontiguousarray(np.asarray(v, np.float32)) for k, v in shared.items()}
    feats = np.ascontiguousarray(np.asarray(object_features, np.float32))

    in_maps = []
    for c in range(B):
        m = {"feat": feats[c].reshape(NTOK, D)}
        m.update(shared)
        m.update(consts)
        in_maps.append(m)

    r = bass_utils.run_bass_kernel_spmd(nc, in_maps, core_ids=list(range(B)),
                                        trace=TRACE)
    LAST_EXEC_TIME_NS = r.exec_time_ns

    bbox_x = np.stack([r.results[c]["o_bbox"].reshape(T, Q, 4) for c in range(B)])
    cls_x = np.stack([r.results[c]["o_cls"].reshape(T, Q, C) for c in range(B)])
    ori = np.stack([r.results[c]["o_ori"].reshape(T, Q, 4) for c in range(B)])
    ids = np.stack([r.results[c]["o_ids"] for c in range(B)]).astype(np.int32)
    return bbox_x, cls_x, ori, ids


# revision 10
# speedup vs baseline: 1.4338x; 1.4338x over previous
"""BBox+Cls decoder TRN2 kernel (data-parallel over batch, 8 NeuronCores).

Per core: one batch element -> 6144 tokens x 512 features through two
3-layer MLP heads (bbox->4, cls->91), sigmoid, cxcywh->xyxy transform and a
closed-form track-ID assignment (the reference's sequential scan reduces to
cumulative sums because IDs, once assigned, never change).

Matmuls run as float32r (1 PE cycle/row at N>=256 vs 4 for fp32; ~1e-4 rel
error measured on HW). Activations stay feature-major ([D, tokens]) so each
layer feeds the next without transposes; the input and the small final-layer
outputs are transposed on the PE. Outputs are sigmoided and DMA'd per
512-token tile so the stores hide under compute.
"""

import numpy as np

import concourse.bass as bass
import concourse.mybir as mybir
import concourse.tile as tile
from concourse import bass_utils

F32 = mybir.dt.float32
F32R = mybir.dt.float32r
I32 = mybir.dt.int32
ALU = mybir.AluOpType
ACTF = mybir.ActivationFunctionType
AX = mybir.AxisListType

B, T, Q, D, H, C = 8, 24, 256, 512, 512, 91
IMG_W, IMG_H = 1088.0, 608.0
P = 128
NTOK = T * Q            # 6144 tokens per core
NTILE = 512             # tokens per n-tile
NT = NTOK // NTILE      # 12 n-tiles
GPT = NTILE // P        # 4 token groups (of 128) per n-tile
G = NTOK // P           # 48 token groups per core
KO = D // P             # 4 contraction chunks

TRACE = False
LAST_EXEC_TIME_NS = None


def _split_multiwait(nc):
    # This walrus build rejects >1 sync-wait per engine instruction; hoist
    # extras onto preceding same-engine NoOps (the sequencer executes waits
    # in program order, so semantics are identical).
    for f in nc.m.functions:
        for blk in f.blocks:
            out, changed = [], False
            for ins in blk.instructions:
                si = ins.sync_info
                if si is not None and len(si.on_wait) > 1:
                    waits = list(si.on_wait)
                    for j, w in enumerate(waits[:-1]):
                        nd = mybir.InstNoOp(name=f"{ins.name}_wsplit{j}", ins=[], outs=[])
                        nd.engine = ins.engine
                        nd.sync_info = mybir.SyncInfo(on_wait=[w], on_update=[])
                        out.append(nd)
                    si.on_wait = waits[-1:]
                    ins.sync_info = si
                    changed = True
                out.append(ins)
            if changed:
                blk.instructions = out


def _build():
    nc = bass.Bass("TRN2", target_bir_lowering=False, debug=False, num_devices=8)

    feat = nc.dram_tensor("feat", [NTOK, D], F32, kind="ExternalInput").ap()
    wd, bd = {}, {}
    for head, fin in (("b", 4), ("c", C)):
        for li, (n, k) in enumerate([(D, H), (H, H), (H, fin)]):
            wd[head, li] = nc.dram_tensor(f"w{li}{head}", [n, k], F32, kind="ExternalInput").ap()
            bd[head, li] = nc.dram_tensor(f"b{li}{head}", [k], F32, kind="ExternalInput").ap()
    ident_d = nc.dram_tensor("ident", [P, P], F32, kind="ExternalInput").ap()
    tri_i_d = nc.dram_tensor("tri_incl", [T, T], F32, kind="ExternalInput").ap()

    o_bbox = nc.dram_tensor("o_bbox", [NTOK, 4], F32, kind="ExternalOutput").ap()
    o_cls = nc.dram_tensor("o_cls", [NTOK, C], F32, kind="ExternalOutput").ap()
    o_ori = nc.dram_tensor("o_ori", [NTOK, 4], F32, kind="ExternalOutput").ap()
    o_ids = nc.dram_tensor("o_ids", [T, Q], I32, kind="ExternalOutput").ap()

    o_bbox_r = o_bbox.rearrange("(g p) c -> p g c", p=P)
    o_cls_r = o_cls.rearrange("(g p) c -> p g c", p=P)
    o_ori_r = o_ori.rearrange("(g p) c -> p g c", p=P)

    with tile.TileContext(nc) as tc:
        with (
            tc.tile_pool(name="const", bufs=1) as cpool,
            tc.tile_pool(name="io", bufs=2) as iopool,
            tc.tile_pool(name="act", bufs=2) as apool,
            tc.tile_pool(name="big", bufs=1) as bigpool,
            tc.tile_pool(name="idp", bufs=1) as idpool,
            tc.tile_pool(name="psT", bufs=2, space="PSUM") as psT,
            tc.tile_pool(name="psH", bufs=2, space="PSUM") as psH,
            tc.tile_pool(name="psF", bufs=2, space="PSUM") as psF,
        ):
            # ---- constants: DMA fp32 then DVE-convert (to f32r where used
            # by f32r matmuls; consumers then sync on the DVE semaphore
            # instead of scattered DMA lanes).
            def conv(dram_ap, shape, dt, name):
                raw = cpool.tile(shape, F32, tag=f"{name}_raw", name=f"{name}_raw")
                nc.sync.dma_start(raw[:], dram_ap)
                t = cpool.tile(shape, dt, tag=name, name=name)
                nc.vector.tensor_copy(t[:], raw[:])
                return t

            w = {}
            for head in ("b", "c"):
                for li in range(2):
                    w[head, li] = conv(
                        wd[head, li].rearrange("(ko p) h -> p ko h", p=P),
                        [P, KO, H], F32R, f"w{li}{head}")
            w["b", 2] = conv(wd["b", 2].rearrange("(ko p) h -> p ko h", p=P),
                             [P, KO, 4], F32R, "w2b")
            w["c", 2] = conv(wd["c", 2].rearrange("(ko p) h -> p ko h", p=P),
                             [P, KO, C], F32R, "w2c")
            bcol = {}
            for head in ("b", "c"):
                for li in range(2):
                    bcol[head, li] = conv(
                        bd[head, li].rearrange("(m p) -> p m", p=P),
                        [P, KO], F32, f"b{li}{head}")
            b3col = {
                "b": conv(bd["b", 2][:, None], [4, 1], F32, "b3b"),
                "c": conv(bd["c", 2][:, None], [C, 1], F32, "b3c"),
            }
            ident = conv(ident_d, [P, P], F32, "ident")
            tri_i = conv(tri_i_d, [T, T], F32, "tri_i")

            # ---- persistent output staging
            clsbuf = bigpool.tile([P, G, C], F32)        # cls logits -> sigmoid in place
            bb_all = bigpool.tile([P, G, 4], F32)        # bbox logits -> sigmoid in place
            ori_all = bigpool.tile([P, G, 4], F32)
            ml = [bigpool.tile([P, T], F32, tag=f"ml{h}", name=f"ml{h}")
                  for h in range(2)]

            feat_r = feat.rearrange("(nt g p) d -> nt p g d", nt=NT, p=P)

            x_tm = [None] * NT
            xT = [None] * NT

            def load_xtm(nt):
                t = iopool.tile([P, GPT, D], F32, tag="x_tm", name="x_tm")
                nc.sync.dma_start(t[:], feat_r[nt])
                x_tm[nt] = t

            def transpose_in(nt):
                # x_tm [tok(P), g, d] -> xT [d(P), ko, tok] via PE transpose
                xt = apool.tile([P, KO, NTILE], F32R, tag="xT", name="xT")
                for g in range(GPT):
                    pt = psT.tile([P, KO * P], F32, name="psTt")
                    for ko in range(KO):
                        nc.tensor.transpose(
                            pt[:, ko * P:(ko + 1) * P],
                            x_tm[nt][:, g, ko * P:(ko + 1) * P],
                            ident[:],
                        )
                    nc.vector.tensor_copy(
                        xt[:, :, g * P:(g + 1) * P],
                        pt[:].rearrange("p (ko q) -> p ko q", ko=KO),
                    )
                xT[nt] = xt

            def mlp_layer(rhs, wt, bias, out_tag):
                # rhs [P, KO, NTILE] f32r; out h^T [P, KO(m), NTILE] f32r
                out = apool.tile([P, KO, NTILE], F32R, tag=out_tag, name=out_tag)
                for m in range(KO):
                    pt = psH.tile([P, NTILE], F32, name="psHt")
                    for ko in range(KO):
                        nc.tensor.matmul(
                            pt[:], wt[:, ko, m * P:(m + 1) * P], rhs[:, ko, :],
                            start=(ko == 0), stop=(ko == KO - 1))
                    if m < 2:
                        nc.vector.tensor_scalar(
                            out[:, m, :], pt[:], bias[:, m:m + 1], 0.0,
                            ALU.add, ALU.max)
                    else:
                        nc.scalar.activation(
                            out[:, m, :], pt[:], ACTF.Relu,
                            bias=bias[:, m:m + 1], scale=1.0)
                return out

            def final_layer(nt, h2, head):
                # feature-major final: fm [fin, ntile] = W3^T @ h2  (f32r,
                # N=512 full speed), bias folded into the PSUM->SBUF copy,
                # then PE transpose-back to token-major 128-tok tiles.
                fin = 4 if head == "b" else C
                fm_ps = psF.tile([fin, NTILE], F32, tag="fm", name="fm_ps")
                for ko in range(KO):
                    nc.tensor.matmul(fm_ps[:], w[head, 2][:, ko, :], h2[:, ko, :],
                                     start=(ko == 0), stop=(ko == KO - 1))
                fm = apool.tile([fin, NTILE], F32, tag=f"fm{head}", name=f"fm{head}")
                nc.vector.tensor_scalar(fm[:], fm_ps[:], b3col[head][:], None, ALU.add)
                for g in range(GPT):
                    j = nt * GPT + g
                    tb = psF.tile([P, fin], F32, tag="tb", name="tb_ps")
                    nc.tensor.transpose(tb[:], fm[:, g * P:(g + 1) * P],
                                        ident[:fin, :fin])
                    if head == "b":
                        nc.vector.tensor_copy(bb_all[:, j, :], tb[:])
                    else:
                        nc.vector.tensor_copy(clsbuf[:, j, :], tb[:])
                        nc.vector.tensor_reduce(
                            ml[j % 2][:, j // 2:j // 2 + 1], clsbuf[:, j, :],
                            axis=AX.X, op=ALU.max)

            uw = idpool.tile([P, GPT], F32, tag="uw")
            uh = idpool.tile([P, GPT], F32, tag="uh")

            def emit_outputs(nt):
                g0, g1 = nt * GPT, (nt + 1) * GPT
                # cls: sigmoid in place, stream out
                cs = clsbuf[:, g0:g1, :]
                nc.scalar.activation(cs.rearrange("p g c -> p (g c)"),
                                     cs.rearrange("p g c -> p (g c)"), ACTF.Sigmoid)
                nc.sync.dma_start(o_cls_r[:, g0:g1, :], cs)
                # bbox: sigmoid in place, compute ori, stream both
                bs = bb_all[:, g0:g1, :]
                nc.scalar.activation(bs.rearrange("p g c -> p (g c)"),
                                     bs.rearrange("p g c -> p (g c)"), ACTF.Sigmoid)
                nc.vector.tensor_scalar(uw[:], bs[:, :, 2], 0.5, None, ALU.mult)
                nc.vector.tensor_scalar(uh[:], bs[:, :, 3], 0.5, None, ALU.mult)
                os_ = ori_all[:, g0:g1, :]
                for col, ci, u, s in ((0, 0, uw, IMG_W), (1, 1, uh, IMG_H),
                                      (2, 0, uw, IMG_W), (3, 1, uh, IMG_H)):
                    op = ALU.subtract if col < 2 else ALU.add
                    nc.vector.tensor_tensor(os_[:, :, col], bs[:, :, ci], u[:], op)
                    nc.vector.tensor_scalar(os_[:, :, col], os_[:, :, col],
                                            s, None, ALU.mult)
                nc.sync.dma_start(o_bbox_r[:, g0:g1, :], bs)
                nc.sync.dma_start(o_ori_r[:, g0:g1, :], os_)

            # ---- main loop
            load_xtm(0)
            transpose_in(0)
            for nt in range(NT):
                if nt + 1 < NT:
                    load_xtm(nt + 1)
                    transpose_in(nt + 1)
                for head in ("b", "c"):
                    h1 = mlp_layer(xT[nt], w[head, 0], bcol[head, 0], f"h1{head}")
                    h2 = mlp_layer(h1, w[head, 1], bcol[head, 1], f"h2{head}")
                    final_layer(nt, h2, head)
                emit_outputs(nt)
                x_tm[nt] = xT[nt] = None

            # ---- track IDs (closed form, [t, q] layout after PE transpose)
            msk = [idpool.tile([P, T], F32, tag=f"msk{h}", name=f"msk{h}")
                   for h in range(2)]
            for h in range(2):
                nc.vector.tensor_scalar(msk[h][:], ml[h][:], 0.0, None, ALU.is_ge)
            m24 = idpool.tile([T, Q], F32, tag="m24")
            for h in range(2):
                pt = psF.tile([T, P], F32, tag="fm", name="psS")
                nc.tensor.transpose(pt[:], msk[h][:], ident[:])
                nc.vector.tensor_copy(m24[:, h * P:(h + 1) * P], pt[:])
            c24p = psF.tile([T, Q], F32, tag="fm", name="psC")
            nc.tensor.matmul(c24p[:], tri_i[:], m24[:], start=True, stop=True)
            vmask = idpool.tile([T, Q], F32, tag="vmask")
            nc.vector.tensor_scalar(vmask[:], c24p[:], 0.5, None, ALU.is_ge)
            born = idpool.tile([T, Q], F32, tag="born")
            nc.vector.tensor_scalar(born[:], c24p[:], 1.0, None, ALU.is_equal)
            nc.vector.tensor_tensor(born[:], born[:], m24[:], ALU.mult)
            # excl[t] = #tracks born before frame t = sum_q [C - M >= 1]
            vprev = idpool.tile([T, Q], F32, tag="vprev")
            nc.vector.tensor_tensor(vprev[:], c24p[:], m24[:], ALU.subtract)
            nc.vector.tensor_scalar(vprev[:], vprev[:], 0.5, None, ALU.is_ge)
            excl = idpool.tile([T, 1], F32, tag="excl")
            nc.vector.tensor_reduce(excl[:], vprev[:], axis=AX.X, op=ALU.add)
            # exclusive prefix over q (shift-add doubling), ping-pong
            qp = [idpool.tile([T, Q], F32, tag=f"qp{i}", name=f"qp{i}")
                  for i in range(2)]
            nc.vector.memset(qp[0][:, :1], 0)
            nc.vector.tensor_copy(qp[0][:, 1:], born[:, :Q - 1])
            cur = 0
            for sh in (1, 2, 4, 8, 16, 32, 64, 128):
                src, dst = qp[cur], qp[1 - cur]
                nc.vector.tensor_copy(dst[:, :sh], src[:, :sh])
                nc.vector.tensor_tensor(dst[:, sh:], src[:, sh:], src[:, :Q - sh], ALU.add)
                cur = 1 - cur
            idval1 = qp[cur]
            nc.vector.tensor_scalar(idval1[:], idval1[:], excl[:], 1.0, ALU.add, ALU.add)
            pick = qp[1 - cur]
            nc.vector.tensor_tensor(pick[:], born[:], idval1[:], ALU.mult)
            # cumsum over t of the one-hot picks -> (ID+1) from t0 onward
            idsp = psF.tile([T, Q], F32, tag="fm", name="psC2")
            nc.tensor.matmul(idsp[:], tri_i[:], pick[:], start=True, stop=True)
            ids_f = idpool.tile([T, Q], F32, tag="ids_f")
            nc.vector.tensor_tensor(ids_f[:], vmask[:], idsp[:], ALU.mult)
            nc.vector.tensor_scalar(ids_f[:], ids_f[:], 1.0, None, ALU.subtract)
            ids_i = idpool.tile([T, Q], I32, tag="ids_i")
            nc.vector.tensor_copy(ids_i[:], ids_f[:])
            nc.sync.dma_start(o_ids, ids_i[:])

    _split_multiwait(nc)
    return nc


_NC = None


def kernel(object_features,
           bbox_w0, bbox_b0, bbox_w1, bbox_b1, bbox_w2, bbox_b2,
           cls_w0, cls_b0, cls_w1, cls_b1, cls_w2, cls_b2):
    global _NC, LAST_EXEC_TIME_NS
    if _NC is None:
        _NC = _build()
    nc = _NC

    consts = {
        "ident": np.eye(P, dtype=np.float32),
        "tri_incl": np.triu(np.ones((T, T), np.float32)),
    }
    shared = {
        "w0b": bbox_w0, "w1b": bbox_w1, "w2b": bbox_w2,
        "b0b": bbox_b0, "b1b": bbox_b1, "b2b": bbox_b2,
        "w0c": cls_w0, "w1c": cls_w1, "w2c": cls_w2,
        "b0c": cls_b0, "b1c": cls_b1, "b2c": cls_b2,
    }
    shared = {k: np.ascontiguousarray(np.asarray(v, np.float32)) for k, v in shared.items()}
    feats = np.ascontiguousarray(np.asarray(object_features, np.float32))

    in_maps = []
    for c in range(B):
        m = {"feat": feats[c].reshape(NTOK, D)}
        m.update(shared)
        m.update(consts)
        in_maps.append(m)

    r = bass_utils.run_bass_kernel_spmd(nc, in_maps, core_ids=list(range(B)),
                                        trace=TRACE)
    LAST_EXEC_TIME_NS = r.exec_time_ns

    bbox_x = np.stack([r.results[c]["o_bbox"].reshape(T, Q, 4) for c in range(B)])
    cls_x = np.stack([r.results[c]["o_cls"].reshape(T, Q, C) for c in range(B)])
    ori = np.stack([r.results[c]["o_ori"].reshape(T, Q, 4) for c in range(B)])
    ids = np.stack([r.results[c]["o_ids"] for c in range(B)]).astype(np.int32)
    return bbox_x, cls_x, ori, ids


# revision 19
# speedup vs baseline: 1.5798x; 1.1018x over previous
"""BBox+Cls decoder TRN2 kernel (data-parallel over batch, 8 NeuronCores).

Per core: one batch element -> 6144 tokens x 512 features through two
3-layer MLP heads (bbox->4, cls->91), sigmoid, cxcywh->xyxy transform and a
closed-form track-ID assignment (the reference's sequential scan reduces to
cumulative sums because IDs, once assigned, never change).

Matmuls run as float32r (1 PE cycle/row at N>=256 vs 4 for fp32; ~1e-4 rel
error measured on HW). Activations stay feature-major ([D, tokens]) so each
layer feeds the next without transposes; the input and the small final-layer
outputs are transposed on the PE. Outputs are sigmoided and DMA'd per
512-token tile so the stores hide under compute.
"""

import numpy as np

import concourse.bass as bass
import concourse.mybir as mybir
import concourse.tile as tile
from concourse import bass_utils

F32 = mybir.dt.float32
F32R = mybir.dt.float32r
I32 = mybir.dt.int32
ALU = mybir.AluOpType
ACTF = mybir.ActivationFunctionType
AX = mybir.AxisListType

B, T, Q, D, H, C = 8, 24, 256, 512, 512, 91
IMG_W, IMG_H = 1088.0, 608.0
P = 128
NTOK = T * Q            # 6144 tokens per core
NTILE = 512             # tokens per n-tile
NT = NTOK // NTILE      # 12 n-tiles
GPT = NTILE // P        # 4 token groups (of 128) per n-tile
G = NTOK // P           # 48 token groups per core
KO = D // P             # 4 contraction chunks

TRACE = False
LAST_EXEC_TIME_NS = None


def _split_multiwait(nc):
    # This walrus build rejects >1 sync-wait per engine instruction; hoist
    # extras onto preceding same-engine NoOps (the sequencer executes waits
    # in program order, so semantics are identical).
    for f in nc.m.functions:
        for blk in f.blocks:
            out, changed = [], False
            for ins in blk.instructions:
                si = ins.sync_info
                if si is not None and len(si.on_wait) > 1:
                    waits = list(si.on_wait)
                    for j, w in enumerate(waits[:-1]):
                        nd = mybir.InstNoOp(name=f"{ins.name}_wsplit{j}", ins=[], outs=[])
                        nd.engine = ins.engine
                        nd.sync_info = mybir.SyncInfo(on_wait=[w], on_update=[])
                        out.append(nd)
                    si.on_wait = waits[-1:]
                    ins.sync_info = si
                    changed = True
                out.append(ins)
            if changed:
                blk.instructions = out


def _build():
    nc = bass.Bass("TRN2", target_bir_lowering=False, debug=False, num_devices=8)

    feat = nc.dram_tensor("feat", [NTOK, D], F32, kind="ExternalInput").ap()
    wd, bd = {}, {}
    for head, fin in (("b", 4), ("c", C)):
        for li, (n, k) in enumerate([(D, H), (H, H), (H, fin)]):
            wd[head, li] = nc.dram_tensor(f"w{li}{head}", [n, k], F32, kind="ExternalInput").ap()
            bd[head, li] = nc.dram_tensor(f"b{li}{head}", [k], F32, kind="ExternalInput").ap()
    ident_d = nc.dram_tensor("ident", [P, P], F32, kind="ExternalInput").ap()
    tri_i_d = nc.dram_tensor("tri_incl", [T, T], F32, kind="ExternalInput").ap()

    o_bbox = nc.dram_tensor("o_bbox", [NTOK, 4], F32, kind="ExternalOutput").ap()
    o_cls = nc.dram_tensor("o_cls", [NTOK, C], F32, kind="ExternalOutput").ap()
    o_ori = nc.dram_tensor("o_ori", [NTOK, 4], F32, kind="ExternalOutput").ap()
    o_ids = nc.dram_tensor("o_ids", [T, Q], I32, kind="ExternalOutput").ap()

    o_bbox_r = o_bbox.rearrange("(g p) c -> p g c", p=P)
    o_cls_r = o_cls.rearrange("(g p) c -> p g c", p=P)
    o_ori_r = o_ori.rearrange("(g p) c -> p g c", p=P)

    with tile.TileContext(nc) as tc:
        with (
            tc.tile_pool(name="const", bufs=1) as cpool,
            tc.tile_pool(name="io", bufs=2) as iopool,
            tc.tile_pool(name="act", bufs=2) as apool,
            tc.tile_pool(name="big", bufs=1) as bigpool,
            tc.tile_pool(name="idp", bufs=1) as idpool,
            tc.tile_pool(name="psT", bufs=2, space="PSUM") as psT,
            tc.tile_pool(name="psH", bufs=3, space="PSUM") as psH,
            tc.tile_pool(name="psF", bufs=2, space="PSUM") as psFf,
            tc.tile_pool(name="psB", bufs=1, space="PSUM") as psB,
        ):
            # ---- constants: DMA fp32 then DVE-convert (to f32r where used
            # by f32r matmuls; consumers then sync on the DVE semaphore
            # instead of scattered DMA lanes).
            def conv(dram_ap, shape, dt, name):
                raw = cpool.tile(shape, F32, tag=f"{name}_raw", name=f"{name}_raw")
                nc.sync.dma_start(raw[:], dram_ap)
                t = cpool.tile(shape, dt, tag=name, name=name)
                nc.vector.tensor_copy(t[:], raw[:])
                return t

            # ---- persistent output staging
            clsbuf = bigpool.tile([P, G, C], F32)        # cls logits -> sigmoid in place
            bb_all = bigpool.tile([P, G, 4], F32)        # bbox logits -> sigmoid in place
            ori_all = bigpool.tile([P, G, 4], F32)
            ml = [bigpool.tile([P, T], F32, tag=f"ml{h}", name=f"ml{h}")
                  for h in range(2)]

            feat_r = feat.rearrange("(nt g p) d -> nt p g d", nt=NT, p=P)

            x_tm = [None] * NT
            xT = [None] * NT

            def load_xtm(nt):
                t = iopool.tile([P, GPT, D], F32, tag="x_tm", name="x_tm")
                nc.sync.dma_start(t[:], feat_r[nt])
                x_tm[nt] = t

            def transpose_in(nt):
                # x_tm [tok(P), g, d] -> xT [d(P), ko, tok] via PE transpose
                xt = apool.tile([P, KO, NTILE], F32R, tag="xT", name="xT")
                for g in range(GPT):
                    pt = psT.tile([P, KO * P], F32, name="psTt")
                    for ko in range(KO):
                        nc.tensor.transpose(
                            pt[:, ko * P:(ko + 1) * P],
                            x_tm[nt][:, g, ko * P:(ko + 1) * P],
                            ident[:],
                        )
                    nc.vector.tensor_copy(
                        xt[:, :, g * P:(g + 1) * P],
                        pt[:].rearrange("p (ko q) -> p ko q", ko=KO),
                    )
                xT[nt] = xt

            def mlp_layer(rhs, wt, bias, out_tag):
                # rhs [P, KO, NTILE] f32r; out h^T [P, KO(m), NTILE] f32r
                out = apool.tile([P, KO, NTILE], F32R, tag=out_tag, name=out_tag)
                for m in range(KO):
                    pt = psH.tile([P, NTILE], F32, name="psHt")
                    for ko in range(KO):
                        nc.tensor.matmul(
                            pt[:], wt[:, ko, m * P:(m + 1) * P], rhs[:, ko, :],
                            start=(ko == 0), stop=(ko == KO - 1))
                    nc.vector.tensor_scalar(
                        out[:, m, :], pt[:], bias[:, m:m + 1], 0.0,
                        ALU.add, ALU.max)
                return out

            def final_layer(nt, h2, head):
                # feature-major final: fm [fin, ntile] = W3^T @ h2  (f32r,
                # N=512 full speed), bias folded into the PSUM->SBUF copy,
                # then PE transpose-back to token-major 128-tok tiles.
                fin = 4 if head == "b" else C
                fm_ps = psB.tile([fin, NTILE], F32, tag="fm", name="fm_ps")
                for ko in range(KO):
                    nc.tensor.matmul(fm_ps[:], w[head, 2][:, ko, :], h2[:, ko, :],
                                     start=(ko == 0), stop=(ko == KO - 1))
                fm = apool.tile([fin, NTILE], F32, tag=f"fm{head}", name=f"fm{head}")
                nc.vector.tensor_scalar(fm[:], fm_ps[:], b3col[head][:], None, ALU.add)
                for g in range(GPT):
                    j = nt * GPT + g
                    tb = psFf.tile([P, fin], F32, tag="tb", name="tb_ps")
                    nc.tensor.transpose(tb[:], fm[:, g * P:(g + 1) * P],
                                        ident[:fin, :fin])
                    if head == "b":
                        nc.vector.tensor_copy(bb_all[:, j, :], tb[:])
                    else:
                        nc.vector.tensor_copy(clsbuf[:, j, :], tb[:])
                        nc.vector.tensor_reduce(
                            ml[j % 2][:, j // 2:j // 2 + 1], clsbuf[:, j, :],
                            axis=AX.X, op=ALU.max)

            uw = idpool.tile([P, GPT], F32, tag="uw")
            uh = idpool.tile([P, GPT], F32, tag="uh")

            def emit_outputs(nt):
                g0, g1 = nt * GPT, (nt + 1) * GPT
                # cls: sigmoid in place, stream out
                cs = clsbuf[:, g0:g1, :]
                nc.scalar.activation(cs.rearrange("p g c -> p (g c)"),
                                     cs.rearrange("p g c -> p (g c)"), ACTF.Sigmoid)
                nc.sync.dma_start(o_cls_r[:, g0:g1, :], cs)
                # bbox: sigmoid in place, compute ori, stream both
                bs = bb_all[:, g0:g1, :]
                nc.scalar.activation(bs.rearrange("p g c -> p (g c)"),
                                     bs.rearrange("p g c -> p (g c)"), ACTF.Sigmoid)
                nc.vector.tensor_scalar(uw[:], bs[:, :, 2], 0.5, None, ALU.mult)
                nc.vector.tensor_scalar(uh[:], bs[:, :, 3], 0.5, None, ALU.mult)
                os_ = ori_all[:, g0:g1, :]
                for col, ci, u, s in ((0, 0, uw, IMG_W), (1, 1, uh, IMG_H),
                                      (2, 0, uw, IMG_W), (3, 1, uh, IMG_H)):
                    op = ALU.subtract if col < 2 else ALU.add
                    nc.vector.tensor_tensor(os_[:, :, col], bs[:, :, ci], u[:], op)
                    nc.vector.tensor_scalar(os_[:, :, col], os_[:, :, col],
                                            s, None, ALU.mult)
                nc.sync.dma_start(o_bbox_r[:, g0:g1, :], bs)
                nc.sync.dma_start(o_ori_r[:, g0:g1, :], os_)

            # ---- preamble, ordered so the first n-tile's dependencies
            # (feat[0], ident, L1 weights) land before the rest of the
            # 4.4MB weight traffic
            load_xtm(0)
            ident = conv(ident_d, [P, P], F32, "ident")
            w, bcol = {}, {}
            w["b", 0] = conv(wd["b", 0].rearrange("(ko p) h -> p ko h", p=P),
                             [P, KO, H], F32R, "w0b")
            bcol["b", 0] = conv(bd["b", 0].rearrange("(m p) -> p m", p=P),
                                [P, KO], F32, "b0b")
            transpose_in(0)
            for head, li in (("b", 1), ("c", 0), ("c", 1)):
                w[head, li] = conv(
                    wd[head, li].rearrange("(ko p) h -> p ko h", p=P),
                    [P, KO, H], F32R, f"w{li}{head}")
                bcol[head, li] = conv(
                    bd[head, li].rearrange("(m p) -> p m", p=P),
                    [P, KO], F32, f"b{li}{head}")
            w["b", 2] = conv(wd["b", 2].rearrange("(ko p) h -> p ko h", p=P),
                             [P, KO, 4], F32R, "w2b")
            w["c", 2] = conv(wd["c", 2].rearrange("(ko p) h -> p ko h", p=P),
                             [P, KO, C], F32R, "w2c")
            b3col = {
                "b": conv(bd["b", 2][:, None], [4, 1], F32, "b3b"),
                "c": conv(bd["c", 2][:, None], [C, 1], F32, "b3c"),
            }
            tri_i = conv(tri_i_d, [T, T], F32, "tri_i")

            # ---- main loop
            for nt in range(NT):
                if nt + 1 < NT:
                    load_xtm(nt + 1)
                    transpose_in(nt + 1)
                for head in ("b", "c"):
                    h1 = mlp_layer(xT[nt], w[head, 0], bcol[head, 0], f"h1{head}")
                    h2 = mlp_layer(h1, w[head, 1], bcol[head, 1], f"h2{head}")
                    final_layer(nt, h2, head)
                emit_outputs(nt)
                x_tm[nt] = xT[nt] = None

            # ---- track IDs (closed form, [t, q] layout after PE transpose)
            msk = [idpool.tile([P, T], F32, tag=f"msk{h}", name=f"msk{h}")
                   for h in range(2)]
            for h in range(2):
                nc.vector.tensor_scalar(msk[h][:], ml[h][:], 0.0, None, ALU.is_ge)
            m24 = idpool.tile([T, Q], F32, tag="m24")
            for h in range(2):
                pt = psB.tile([T, P], F32, tag="fm", name="psS")
                nc.tensor.transpose(pt[:], msk[h][:], ident[:])
                nc.vector.tensor_copy(m24[:, h * P:(h + 1) * P], pt[:])
            c24p = psB.tile([T, Q], F32, tag="fm", name="psC")
            nc.tensor.matmul(c24p[:], tri_i[:], m24[:], start=True, stop=True)
            vmask = idpool.tile([T, Q], F32, tag="vmask")
            nc.vector.tensor_scalar(vmask[:], c24p[:], 0.5, None, ALU.is_ge)
            born = idpool.tile([T, Q], F32, tag="born")
            nc.vector.tensor_scalar(born[:], c24p[:], 1.0, None, ALU.is_equal)
            nc.vector.tensor_tensor(born[:], born[:], m24[:], ALU.mult)
            # excl[t] = #tracks born before frame t = sum_q [C - M >= 1]
            vprev = idpool.tile([T, Q], F32, tag="vprev")
            nc.vector.tensor_tensor(vprev[:], c24p[:], m24[:], ALU.subtract)
            nc.vector.tensor_scalar(vprev[:], vprev[:], 0.5, None, ALU.is_ge)
            excl = idpool.tile([T, 1], F32, tag="excl")
            nc.vector.tensor_reduce(excl[:], vprev[:], axis=AX.X, op=ALU.add)
            # exclusive prefix over q (shift-add doubling), ping-pong
            qp = [idpool.tile([T, Q], F32, tag=f"qp{i}", name=f"qp{i}")
                  for i in range(2)]
            nc.vector.memset(qp[0][:, :1], 0)
            nc.vector.tensor_copy(qp[0][:, 1:], born[:, :Q - 1])
            cur = 0
            for sh in (1, 2, 4, 8, 16, 32, 64, 128):
                src, dst = qp[cur], qp[1 - cur]
                nc.vector.tensor_copy(dst[:, :sh], src[:, :sh])
                nc.vector.tensor_tensor(dst[:, sh:], src[:, sh:], src[:, :Q - sh], ALU.add)
                cur = 1 - cur
            idval1 = qp[cur]
            nc.vector.tensor_scalar(idval1[:], idval1[:], excl[:], 1.0, ALU.add, ALU.add)
            pick = qp[1 - cur]
            nc.vector.tensor_tensor(pick[:], born[:], idval1[:], ALU.mult)
            # cumsum over t of the one-hot picks -> (ID+1) from t0 onward
            idsp = psB.tile([T, Q], F32, tag="fm", name="psC2")
            nc.tensor.matmul(idsp[:], tri_i[:], pick[:], start=True, stop=True)
            ids_f = idpool.tile([T, Q], F32, tag="ids_f")
            nc.vector.tensor_tensor(ids_f[:], vmask[:], idsp[:], ALU.mult)
            nc.vector.tensor_scalar(ids_f[:], ids_f[:], 1.0, None, ALU.subtract)
            ids_i = idpool.tile([T, Q], I32, tag="ids_i")
            nc.vector.tensor_copy(ids_i[:], ids_f[:])
            nc.sync.dma_start(o_ids, ids_i[:])

    _split_multiwait(nc)
    return nc


_NC = None


def kernel(object_features,
           bbox_w0, bbox_b0, bbox_w1, bbox_b1, bbox_w2, bbox_b2,
           cls_w0, cls_b0, cls_w1, cls_b1, cls_w2, cls_b2):
    global _NC, LAST_EXEC_TIME_NS
    if _NC is None:
        _NC = _build()
    nc = _NC

    consts = {
        "ident": np.eye(P, dtype=np.float32),
        "tri_incl": np.triu(np.ones((T, T), np.float32)),
    }
    shared = {
        "w0b": bbox_w0, "w1b": bbox_w1, "w2b": bbox_w2,
        "b0b": bbox_b0, "b1b": bbox_b1, "b2b": bbox_b2,
        "w0c": cls_w0, "w1c": cls_w1, "w2c": cls_w2,
        "b0c": cls_b0, "b1c": cls_b1, "b2c": cls_b2,
    }
    shared = {k: np.ascontiguousarray(np.asarray(v, np.float32)) for k, v in shared.items()}
    feats = np.ascontiguousarray(np.asarray(object_features, np.float32))

    in_maps = []
    for c in range(B):
        m = {"feat": feats[c].reshape(NTOK, D)}
        m.update(shared)
        m.update(consts)
        in_maps.append(m)

    r = bass_utils.run_bass_kernel_spmd(nc, in_maps, core_ids=list(range(B)),
                                        trace=TRACE)
    LAST_EXEC_TIME_NS = r.exec_time_ns

    bbox_x = np.stack([r.results[c]["o_bbox"].reshape(T, Q, 4) for c in range(B)])
    cls_x = np.stack([r.results[c]["o_cls"].reshape(T, Q, C) for c in range(B)])
    ori = np.stack([r.results[c]["o_ori"].reshape(T, Q, 4) for c in range(B)])
    ids = np.stack([r.results[c]["o_ids"] for c in range(B)]).astype(np.int32)
    return bbox_x, cls_x, ori, ids


# revision 24
# speedup vs baseline: 1.6119x; 1.0204x over previous
"""BBox+Cls decoder TRN2 kernel (data-parallel over batch, 8 NeuronCores).

Per core: one batch element -> 6144 tokens x 512 features through two
3-layer MLP heads (bbox->4, cls->91), sigmoid, cxcywh->xyxy transform and a
closed-form track-ID assignment (the reference's sequential scan reduces to
cumulative sums because IDs, once assigned, never change).

Matmuls run as float32r (1 PE cycle/row at N>=256 vs 4 for fp32; ~1e-4 rel
error measured on HW). Activations stay feature-major ([D, tokens]) so each
layer feeds the next without transposes; the input and the small final-layer
outputs are transposed on the PE. Outputs are sigmoided and DMA'd per
512-token tile so the stores hide under compute.
"""

import numpy as np

import concourse.bass as bass
import concourse.mybir as mybir
import concourse.tile as tile
from concourse import bass_utils

F32 = mybir.dt.float32
F32R = mybir.dt.float32r
I32 = mybir.dt.int32
ALU = mybir.AluOpType
ACTF = mybir.ActivationFunctionType
AX = mybir.AxisListType

B, T, Q, D, H, C = 8, 24, 256, 512, 512, 91
IMG_W, IMG_H = 1088.0, 608.0
P = 128
NTOK = T * Q            # 6144 tokens per core
NTILE = 512             # tokens per n-tile
NT = NTOK // NTILE      # 12 n-tiles
GPT = NTILE // P        # 4 token groups (of 128) per n-tile
G = NTOK // P           # 48 token groups per core
KO = D // P             # 4 contraction chunks

TRACE = False
LAST_EXEC_TIME_NS = None


def _split_multiwait(nc):
    # This walrus build rejects >1 sync-wait per engine instruction; hoist
    # extras onto preceding same-engine NoOps (the sequencer executes waits
    # in program order, so semantics are identical).
    for f in nc.m.functions:
        for blk in f.blocks:
            out, changed = [], False
            for ins in blk.instructions:
                si = ins.sync_info
                if si is not None and len(si.on_wait) > 1:
                    waits = list(si.on_wait)
                    for j, w in enumerate(waits[:-1]):
                        nd = mybir.InstNoOp(name=f"{ins.name}_wsplit{j}", ins=[], outs=[])
                        nd.engine = ins.engine
                        nd.sync_info = mybir.SyncInfo(on_wait=[w], on_update=[])
                        out.append(nd)
                    si.on_wait = waits[-1:]
                    ins.sync_info = si
                    changed = True
                out.append(ins)
            if changed:
                blk.instructions = out


def _build():
    nc = bass.Bass("TRN2", target_bir_lowering=False, debug=False, num_devices=8)

    feat = nc.dram_tensor("feat", [NTOK, D], F32, kind="ExternalInput").ap()
    wd, bd = {}, {}
    for head, fin in (("b", 4), ("c", C)):
        for li, (n, k) in enumerate([(D, H), (H, H), (H, fin)]):
            wd[head, li] = nc.dram_tensor(f"w{li}{head}", [n, k], F32, kind="ExternalInput").ap()
            bd[head, li] = nc.dram_tensor(f"b{li}{head}", [k], F32, kind="ExternalInput").ap()
    ident_d = nc.dram_tensor("ident", [P, P], F32, kind="ExternalInput").ap()
    tri_i_d = nc.dram_tensor("tri_incl", [T, T], F32, kind="ExternalInput").ap()

    o_bbox = nc.dram_tensor("o_bbox", [NTOK, 4], F32, kind="ExternalOutput").ap()
    o_cls = nc.dram_tensor("o_cls", [NTOK, C], F32, kind="ExternalOutput").ap()
    o_ori = nc.dram_tensor("o_ori", [NTOK, 4], F32, kind="ExternalOutput").ap()
    o_ids = nc.dram_tensor("o_ids", [T, Q], I32, kind="ExternalOutput").ap()

    o_bbox_r = o_bbox.rearrange("(g p) c -> p g c", p=P)
    o_cls_r = o_cls.rearrange("(g p) c -> p g c", p=P)
    o_ori_r = o_ori.rearrange("(g p) c -> p g c", p=P)

    with tile.TileContext(nc) as tc:
        with (
            tc.tile_pool(name="const", bufs=1) as cpool,
            tc.tile_pool(name="io", bufs=2) as iopool,
            tc.tile_pool(name="act", bufs=2) as apool,
            tc.tile_pool(name="big", bufs=1) as bigpool,
            tc.tile_pool(name="idp", bufs=1) as idpool,
            tc.tile_pool(name="psT", bufs=2, space="PSUM") as psT,
            tc.tile_pool(name="psH", bufs=4, space="PSUM") as psH,
            tc.tile_pool(name="psF", bufs=1, space="PSUM") as psFf,
            tc.tile_pool(name="psB", bufs=1, space="PSUM") as psB,
        ):
            # ---- constants: DMA fp32 then DVE-convert (to f32r where used
            # by f32r matmuls; consumers then sync on the DVE semaphore
            # instead of scattered DMA lanes).
            def conv(dram_ap, shape, dt, name):
                raw = cpool.tile(shape, F32, tag=f"{name}_raw", name=f"{name}_raw")
                nc.sync.dma_start(raw[:], dram_ap)
                t = cpool.tile(shape, dt, tag=name, name=name)
                nc.vector.tensor_copy(t[:], raw[:])
                return t

            # ---- persistent output staging
            clsbuf = bigpool.tile([P, G, C], F32)        # cls logits -> sigmoid in place
            bb_all = bigpool.tile([P, G, 4], F32)        # bbox logits -> sigmoid in place
            ori_all = bigpool.tile([P, G, 4], F32)
            ml = [bigpool.tile([P, T], F32, tag=f"ml{h}", name=f"ml{h}")
                  for h in range(2)]

            feat_r = feat.rearrange("(nt g p) d -> nt p g d", nt=NT, p=P)

            x_tm = [None] * NT
            xT = [None] * NT

            def load_xtm(nt):
                t = iopool.tile([P, GPT, D], F32, tag="x_tm", name="x_tm")
                nc.sync.dma_start(t[:], feat_r[nt])
                x_tm[nt] = t

            def transpose_in(nt):
                # x_tm [tok(P), g, d] -> xT [d(P), ko, tok] via PE transpose
                xt = apool.tile([P, KO, NTILE], F32R, tag="xT", name="xT")
                for g in range(GPT):
                    pt = psT.tile([P, KO * P], F32, name="psTt")
                    for ko in range(KO):
                        nc.tensor.transpose(
                            pt[:, ko * P:(ko + 1) * P],
                            x_tm[nt][:, g, ko * P:(ko + 1) * P],
                            ident[:],
                        )
                    nc.scalar.copy(
                        xt[:, :, g * P:(g + 1) * P],
                        pt[:].rearrange("p (ko q) -> p ko q", ko=KO),
                    )
                xT[nt] = xt

            def mlp_layer(rhs, wt, bias, out_tag):
                # rhs [P, KO, NTILE] f32r; out h^T [P, KO(m), NTILE] f32r
                out = apool.tile([P, KO, NTILE], F32R, tag=out_tag, name=out_tag)
                for m in range(KO):
                    pt = psH.tile([P, NTILE], F32, name="psHt")
                    for ko in range(KO):
                        nc.tensor.matmul(
                            pt[:], wt[:, ko, m * P:(m + 1) * P], rhs[:, ko, :],
                            start=(ko == 0), stop=(ko == KO - 1))
                    if m < 2:
                        nc.vector.tensor_scalar(
                            out[:, m, :], pt[:], bias[:, m:m + 1], 0.0,
                            ALU.add, ALU.max)
                    else:
                        nc.scalar.activation(
                            out[:, m, :], pt[:], ACTF.Relu,
                            bias=bias[:, m:m + 1], scale=1.0)
                return out

            def final_layer(nt, h2, head):
                # feature-major final: fm [fin, ntile] = W3^T @ h2  (f32r,
                # N=512 full speed), bias folded into the PSUM->SBUF copy,
                # then PE transpose-back to token-major 128-tok tiles.
                fin = 4 if head == "b" else C
                fm_ps = psB.tile([fin, NTILE], F32, tag="fm", name="fm_ps")
                for ko in range(KO):
                    nc.tensor.matmul(fm_ps[:], w[head, 2][:, ko, :], h2[:, ko, :],
                                     start=(ko == 0), stop=(ko == KO - 1))
                fm = apool.tile([fin, NTILE], F32, tag=f"fm{head}", name=f"fm{head}")
                nc.scalar.add(fm[:], fm_ps[:], b3col[head][:])
                for g in range(GPT):
                    j = nt * GPT + g
                    tb = psFf.tile([P, fin], F32, tag="tb", name="tb_ps")
                    nc.tensor.transpose(tb[:], fm[:, g * P:(g + 1) * P],
                                        ident[:fin, :fin])
                    if head == "b":
                        nc.vector.tensor_copy(bb_all[:, j, :], tb[:])
                    else:
                        nc.vector.tensor_copy(clsbuf[:, j, :], tb[:])
                        nc.vector.tensor_reduce(
                            ml[j % 2][:, j // 2:j // 2 + 1], clsbuf[:, j, :],
                            axis=AX.X, op=ALU.max)

            uw = idpool.tile([P, GPT], F32, tag="uw")
            uh = idpool.tile([P, GPT], F32, tag="uh")

            def emit_outputs(nt):
                g0, g1 = nt * GPT, (nt + 1) * GPT
                # cls: sigmoid in place, stream out
                cs = clsbuf[:, g0:g1, :]
                nc.scalar.activation(cs.rearrange("p g c -> p (g c)"),
                                     cs.rearrange("p g c -> p (g c)"), ACTF.Sigmoid)
                nc.sync.dma_start(o_cls_r[:, g0:g1, :], cs)
                # bbox: sigmoid in place, compute ori, stream both
                bs = bb_all[:, g0:g1, :]
                nc.scalar.activation(bs.rearrange("p g c -> p (g c)"),
                                     bs.rearrange("p g c -> p (g c)"), ACTF.Sigmoid)
                nc.vector.tensor_scalar(uw[:], bs[:, :, 2], 0.5, None, ALU.mult)
                nc.vector.tensor_scalar(uh[:], bs[:, :, 3], 0.5, None, ALU.mult)
                os_ = ori_all[:, g0:g1, :]
                for col, ci, u, s in ((0, 0, uw, IMG_W), (1, 1, uh, IMG_H),
                                      (2, 0, uw, IMG_W), (3, 1, uh, IMG_H)):
                    op = ALU.subtract if col < 2 else ALU.add
                    nc.vector.tensor_tensor(os_[:, :, col], bs[:, :, ci], u[:], op)
                    nc.vector.tensor_scalar(os_[:, :, col], os_[:, :, col],
                                            s, None, ALU.mult)
                nc.sync.dma_start(o_bbox_r[:, g0:g1, :], bs)
                nc.sync.dma_start(o_ori_r[:, g0:g1, :], os_)

            # ---- preamble, ordered so the first n-tile's dependencies
            # (feat[0], ident, L1 weights) land before the rest of the
            # 4.4MB weight traffic
            load_xtm(0)
            ident = conv(ident_d, [P, P], F32, "ident")
            w, bcol = {}, {}
            w["b", 0] = conv(wd["b", 0].rearrange("(ko p) h -> p ko h", p=P),
                             [P, KO, H], F32R, "w0b")
            bcol["b", 0] = conv(bd["b", 0].rearrange("(m p) -> p m", p=P),
                                [P, KO], F32, "b0b")
            transpose_in(0)
            for head, li in (("b", 1), ("c", 0), ("c", 1)):
                w[head, li] = conv(
                    wd[head, li].rearrange("(ko p) h -> p ko h", p=P),
                    [P, KO, H], F32R, f"w{li}{head}")
                bcol[head, li] = conv(
                    bd[head, li].rearrange("(m p) -> p m", p=P),
                    [P, KO], F32, f"b{li}{head}")
            w["b", 2] = conv(wd["b", 2].rearrange("(ko p) h -> p ko h", p=P),
                             [P, KO, 4], F32R, "w2b")
            w["c", 2] = conv(wd["c", 2].rearrange("(ko p) h -> p ko h", p=P),
                             [P, KO, C], F32R, "w2c")
            b3col = {
                "b": conv(bd["b", 2][:, None], [4, 1], F32, "b3b"),
                "c": conv(bd["c", 2][:, None], [C, 1], F32, "b3c"),
            }
            tri_i = conv(tri_i_d, [T, T], F32, "tri_i")

            # ---- main loop
            for nt in range(NT):
                if nt + 1 < NT:
                    load_xtm(nt + 1)
                    transpose_in(nt + 1)
                h1b = mlp_layer(xT[nt], w["b", 0], bcol["b", 0], "h1b")
                h1c = mlp_layer(xT[nt], w["c", 0], bcol["c", 0], "h1c")
                h2b = mlp_layer(h1b, w["b", 1], bcol["b", 1], "h2b")
                h2c = mlp_layer(h1c, w["c", 1], bcol["c", 1], "h2c")
                final_layer(nt, h2b, "b")
                final_layer(nt, h2c, "c")
                emit_outputs(nt)
                x_tm[nt] = xT[nt] = None

            # ---- track IDs (closed form, [t, q] layout after PE transpose)
            msk = [idpool.tile([P, T], F32, tag=f"msk{h}", name=f"msk{h}")
                   for h in range(2)]
            for h in range(2):
                nc.vector.tensor_scalar(msk[h][:], ml[h][:], 0.0, None, ALU.is_ge)
            m24 = idpool.tile([T, Q], F32, tag="m24")
            for h in range(2):
                pt = psB.tile([T, P], F32, tag="fm", name="psS")
                nc.tensor.transpose(pt[:], msk[h][:], ident[:])
                nc.vector.tensor_copy(m24[:, h * P:(h + 1) * P], pt[:])
            c24p = psB.tile([T, Q], F32, tag="fm", name="psC")
            nc.tensor.matmul(c24p[:], tri_i[:], m24[:], start=True, stop=True)
            vmask = idpool.tile([T, Q], F32, tag="vmask")
            nc.vector.tensor_scalar(vmask[:], c24p[:], 0.5, None, ALU.is_ge)
            born = idpool.tile([T, Q], F32, tag="born")
            nc.vector.tensor_scalar(born[:], c24p[:], 1.0, None, ALU.is_equal)
            nc.vector.tensor_tensor(born[:], born[:], m24[:], ALU.mult)
            # excl[t] = #tracks born before frame t = sum_q [C - M >= 1]
            vprev = idpool.tile([T, Q], F32, tag="vprev")
            nc.vector.tensor_tensor(vprev[:], c24p[:], m24[:], ALU.subtract)
            nc.vector.tensor_scalar(vprev[:], vprev[:], 0.5, None, ALU.is_ge)
            excl = idpool.tile([T, 1], F32, tag="excl")
            nc.vector.tensor_reduce(excl[:], vprev[:], axis=AX.X, op=ALU.add)
            # exclusive prefix over q (shift-add doubling), ping-pong
            qp = [idpool.tile([T, Q], F32, tag=f"qp{i}", name=f"qp{i}")
                  for i in range(2)]
            nc.vector.memset(qp[0][:, :1], 0)
            nc.vector.tensor_copy(qp[0][:, 1:], born[:, :Q - 1])
            cur = 0
            for sh in (1, 2, 4, 8, 16, 32, 64, 128):
                src, dst = qp[cur], qp[1 - cur]
                nc.vector.tensor_copy(dst[:, :sh], src[:, :sh])
                nc.vector.tensor_tensor(dst[:, sh:], src[:, sh:], src[:, :Q - sh], ALU.add)
                cur = 1 - cur
            idval1 = qp[cur]
            nc.vector.tensor_scalar(idval1[:], idval1[:], excl[:], 1.0, ALU.add, ALU.add)
            pick = qp[1 - cur]
            nc.vector.tensor_tensor(pick[:], born[:], idval1[:], ALU.mult)
            # cumsum over t of the one-hot picks -> (ID+1) from t0 onward
            idsp = psB.tile([T, Q], F32, tag="fm", name="psC2")
            nc.tensor.matmul(idsp[:], tri_i[:], pick[:], start=True, stop=True)
            ids_f = idpool.tile([T, Q], F32, tag="ids_f")
            nc.vector.tensor_tensor(ids_f[:], vmask[:], idsp[:], ALU.mult)
            nc.vector.tensor_scalar(ids_f[:], ids_f[:], 1.0, None, ALU.subtract)
            ids_i = idpool.tile([T, Q], I32, tag="ids_i")
            nc.vector.tensor_copy(ids_i[:], ids_f[:])
            nc.sync.dma_start(o_ids, ids_i[:])

    _split_multiwait(nc)
    return nc


_NC = None


def kernel(object_features,
           bbox_w0, bbox_b0, bbox_w1, bbox_b1, bbox_w2, bbox_b2,
           cls_w0, cls_b0, cls_w1, cls_b1, cls_w2, cls_b2):
    global _NC, LAST_EXEC_TIME_NS
    if _NC is None:
        _NC = _build()
    nc = _NC

    consts = {
        "ident": np.eye(P, dtype=np.float32),
        "tri_incl": np.triu(np.ones((T, T), np.float32)),
    }
    shared = {
        "w0b": bbox_w0, "w1b": bbox_w1, "w2b": bbox_w2,
        "b0b": bbox_b0, "b1b": bbox_b1, "b2b": bbox_b2,
        "w0c": cls_w0, "w1c": cls_w1, "w2c": cls_w2,
        "b0c": cls_b0, "b1c": cls_b1, "b2c": cls_b2,
    }
    shared = {k: np.ascontiguousarray(np.asarray(v, np.float32)) for k, v in shared.items()}
    feats = np.ascontiguousarray(np.asarray(object_features, np.float32))

    in_maps = []
    for c in range(B):
        m = {"feat": feats[c].reshape(NTOK, D)}
        m.update(shared)
        m.update(consts)
        in_maps.append(m)

    r = bass_utils.run_bass_kernel_spmd(nc, in_maps, core_ids=list(range(B)),
                                        trace=TRACE)
    LAST_EXEC_TIME_NS = r.exec_time_ns

    bbox_x = np.stack([r.results[c]["o_bbox"].reshape(T, Q, 4) for c in range(B)])
    cls_x = np.stack([r.results[c]["o_cls"].reshape(T, Q, C) for c in range(B)])
    ori = np.stack([r.results[c]["o_ori"].reshape(T, Q, 4) for c in range(B)])
    ids = np.stack([r.results[c]["o_ids"] for c in range(B)]).astype(np.int32)
    return bbox_x, cls_x, ori, ids


# revision 28
# speedup vs baseline: 1.6608x; 1.0303x over previous
"""BBox+Cls decoder TRN2 kernel (data-parallel over batch, 8 NeuronCores).

Per core: one batch element -> 6144 tokens x 512 features through two
3-layer MLP heads (bbox->4, cls->91), sigmoid, cxcywh->xyxy transform and a
closed-form track-ID assignment (the reference's sequential scan reduces to
cumulative sums because IDs, once assigned, never change).

Matmuls run as float32r (1 PE cycle/row at N>=256 vs 4 for fp32; ~1e-4 rel
error measured on HW). Activations stay feature-major ([D, tokens]) so each
layer feeds the next without transposes; the input and the small final-layer
outputs are transposed on the PE. Outputs are sigmoided and DMA'd per
512-token tile so the stores hide under compute.
"""

import numpy as np

import concourse.bass as bass
import concourse.mybir as mybir
import concourse.tile as tile
from concourse import bass_utils

F32 = mybir.dt.float32
F32R = mybir.dt.float32r
I32 = mybir.dt.int32
ALU = mybir.AluOpType
ACTF = mybir.ActivationFunctionType
AX = mybir.AxisListType

B, T, Q, D, H, C = 8, 24, 256, 512, 512, 91
IMG_W, IMG_H = 1088.0, 608.0
P = 128
NTOK = T * Q            # 6144 tokens per core
NTILE = 512             # tokens per n-tile
NT = NTOK // NTILE      # 12 n-tiles
GPT = NTILE // P        # 4 token groups (of 128) per n-tile
G = NTOK // P           # 48 token groups per core
KO = D // P             # 4 contraction chunks

TRACE = False
LAST_EXEC_TIME_NS = None


def _split_multiwait(nc):
    # This walrus build rejects >1 sync-wait per engine instruction; hoist
    # extras onto preceding same-engine NoOps (the sequencer executes waits
    # in program order, so semantics are identical).
    for f in nc.m.functions:
        for blk in f.blocks:
            out, changed = [], False
            for ins in blk.instructions:
                si = ins.sync_info
                if si is not None and len(si.on_wait) > 1:
                    waits = list(si.on_wait)
                    for j, w in enumerate(waits[:-1]):
                        nd = mybir.InstNoOp(name=f"{ins.name}_wsplit{j}", ins=[], outs=[])
                        nd.engine = ins.engine
                        nd.sync_info = mybir.SyncInfo(on_wait=[w], on_update=[])
                        out.append(nd)
                    si.on_wait = waits[-1:]
                    ins.sync_info = si
                    changed = True
                out.append(ins)
            if changed:
                blk.instructions = out


def _build():
    nc = bass.Bass("TRN2", target_bir_lowering=False, debug=False, num_devices=8)

    feat = nc.dram_tensor("feat", [NTOK, D], F32, kind="ExternalInput").ap()
    wd, bd = {}, {}
    for head, fin in (("b", 4), ("c", C)):
        for li, (n, k) in enumerate([(D, H), (H, H), (H, fin)]):
            wd[head, li] = nc.dram_tensor(f"w{li}{head}", [n, k], F32, kind="ExternalInput").ap()
            bd[head, li] = nc.dram_tensor(f"b{li}{head}", [k], F32, kind="ExternalInput").ap()
    ident_d = nc.dram_tensor("ident", [P, P], F32, kind="ExternalInput").ap()
    tri_i_d = nc.dram_tensor("tri_incl", [T, T], F32, kind="ExternalInput").ap()

    o_bbox = nc.dram_tensor("o_bbox", [NTOK, 4], F32, kind="ExternalOutput").ap()
    o_cls = nc.dram_tensor("o_cls", [NTOK, C], F32, kind="ExternalOutput").ap()
    o_ori = nc.dram_tensor("o_ori", [NTOK, 4], F32, kind="ExternalOutput").ap()
    o_ids = nc.dram_tensor("o_ids", [T, Q], I32, kind="ExternalOutput").ap()

    o_bbox_r = o_bbox.rearrange("(g p) c -> p g c", p=P)
    o_cls_r = o_cls.rearrange("(g p) c -> p g c", p=P)
    o_ori_r = o_ori.rearrange("(g p) c -> p g c", p=P)

    with tile.TileContext(nc) as tc:
        with (
            tc.tile_pool(name="const", bufs=1) as cpool,
            tc.tile_pool(name="io", bufs=2) as iopool,
            tc.tile_pool(name="act", bufs=2) as apool,
            tc.tile_pool(name="big", bufs=1) as bigpool,
            tc.tile_pool(name="idp", bufs=1) as idpool,
            tc.tile_pool(name="psT", bufs=2, space="PSUM") as psT,
            tc.tile_pool(name="psH", bufs=4, space="PSUM") as psH,
            tc.tile_pool(name="psB", bufs=2, space="PSUM") as psB,
        ):
            # ---- constants: DMA fp32 then DVE-convert (to f32r where used
            # by f32r matmuls; consumers then sync on the DVE semaphore
            # instead of scattered DMA lanes).
            def conv(dram_ap, shape, dt, name):
                raw = cpool.tile(shape, F32, tag=f"{name}_raw", name=f"{name}_raw")
                nc.sync.dma_start(raw[:], dram_ap)
                t = cpool.tile(shape, dt, tag=name, name=name)
                nc.vector.tensor_copy(t[:], raw[:])
                return t

            # ---- persistent output staging
            clsbuf = bigpool.tile([P, G, C], F32)        # cls logits -> sigmoid in place
            bb_all = bigpool.tile([P, G, 4], F32)        # bbox logits -> sigmoid in place
            ori_all = bigpool.tile([P, G, 4], F32)
            ml = [bigpool.tile([P, T], F32, tag=f"ml{h}", name=f"ml{h}")
                  for h in range(2)]

            feat_r = feat.rearrange("(nt g p) d -> nt p g d", nt=NT, p=P)

            x_tm = [None] * NT
            xT = [None] * NT

            def load_xtm(nt):
                t = iopool.tile([P, GPT, D], F32, tag="x_tm", name="x_tm")
                nc.sync.dma_start(t[:], feat_r[nt])
                x_tm[nt] = t

            def transpose_in(nt):
                # x_tm [tok(P), g, d] -> xT [d(P), ko, tok] via PE transpose
                xt = apool.tile([P, KO, NTILE], F32R, tag="xT", name="xT")
                for g in range(GPT):
                    pt = psT.tile([P, KO * P], F32, name="psTt")
                    for ko in range(KO):
                        nc.tensor.transpose(
                            pt[:, ko * P:(ko + 1) * P],
                            x_tm[nt][:, g, ko * P:(ko + 1) * P],
                            ident[:],
                        )
                    nc.scalar.copy(
                        xt[:, :, g * P:(g + 1) * P],
                        pt[:].rearrange("p (ko q) -> p ko q", ko=KO),
                    )
                xT[nt] = xt

            def mlp_layer(rhs, wt, bias, out_tag):
                # rhs [P, KO, NTILE] f32r; out h^T [P, KO(m), NTILE] f32r
                out = apool.tile([P, KO, NTILE], F32R, tag=out_tag, name=out_tag)
                for m in range(KO):
                    pt = psH.tile([P, NTILE], F32, name="psHt")
                    for ko in range(KO):
                        nc.tensor.matmul(
                            pt[:], wt[:, ko, m * P:(m + 1) * P], rhs[:, ko, :],
                            start=(ko == 0), stop=(ko == KO - 1))
                    if m < 2:
                        nc.vector.tensor_scalar(
                            out[:, m, :], pt[:], bias[:, m:m + 1], 0.0,
                            ALU.add, ALU.max)
                    else:
                        nc.scalar.activation(
                            out[:, m, :], pt[:], ACTF.Relu,
                            bias=bias[:, m:m + 1], scale=1.0)
                return out

            def fm_stage(nt, h2, head):
                # feature-major final: fm [fin, ntile] = W3^T @ h2  (f32r,
                # N=512 full speed), bias folded into the PSUM->SBUF copy
                fin = 4 if head == "b" else C
                fm_ps = psB.tile([fin, NTILE], F32, tag="fm", name="fm_ps")
                for ko in range(KO):
                    nc.tensor.matmul(fm_ps[:], w[head, 2][:, ko, :], h2[:, ko, :],
                                     start=(ko == 0), stop=(ko == KO - 1))
                fm = apool.tile([fin, NTILE], F32, tag=f"fm{head}", name=f"fm{head}")
                nc.scalar.add(fm[:], fm_ps[:], b3col[head][:])
                return fm

            def tb_stage(nt, fm, head):
                # PE transpose-back to token-major 128-token tiles
                fin = 4 if head == "b" else C
                for g in range(GPT):
                    j = nt * GPT + g
                    tb = psT.tile([P, fin], F32, tag="psTt", name="tb_ps")
                    nc.tensor.transpose(tb[:], fm[:, g * P:(g + 1) * P],
                                        ident[:fin, :fin])
                    if head == "b":
                        nc.vector.tensor_copy(bb_all[:, j, :], tb[:])
                    else:
                        nc.vector.tensor_copy(clsbuf[:, j, :], tb[:])
                        nc.vector.tensor_reduce(
                            ml[j % 2][:, j // 2:j // 2 + 1], clsbuf[:, j, :],
                            axis=AX.X, op=ALU.max)

            uw = idpool.tile([P, GPT], F32, tag="uw")
            uh = idpool.tile([P, GPT], F32, tag="uh")

            def emit_outputs(nt):
                g0, g1 = nt * GPT, (nt + 1) * GPT
                # cls: sigmoid in place, stream out
                cs = clsbuf[:, g0:g1, :]
                nc.scalar.activation(cs.rearrange("p g c -> p (g c)"),
                                     cs.rearrange("p g c -> p (g c)"), ACTF.Sigmoid)
                nc.sync.dma_start(o_cls_r[:, g0:g1, :], cs)
                # bbox: sigmoid in place, compute ori, stream both
                bs = bb_all[:, g0:g1, :]
                nc.scalar.activation(bs.rearrange("p g c -> p (g c)"),
                                     bs.rearrange("p g c -> p (g c)"), ACTF.Sigmoid)
                nc.vector.tensor_scalar(uw[:], bs[:, :, 2], 0.5, None, ALU.mult)
                nc.vector.tensor_scalar(uh[:], bs[:, :, 3], 0.5, None, ALU.mult)
                os_ = ori_all[:, g0:g1, :]
                for col, ci, u, s in ((0, 0, uw, IMG_W), (1, 1, uh, IMG_H),
                                      (2, 0, uw, IMG_W), (3, 1, uh, IMG_H)):
                    op = ALU.subtract if col < 2 else ALU.add
                    nc.vector.tensor_tensor(os_[:, :, col], bs[:, :, ci], u[:], op)
                    nc.vector.tensor_scalar(os_[:, :, col], os_[:, :, col],
                                            s, None, ALU.mult)
                nc.sync.dma_start(o_bbox_r[:, g0:g1, :], bs)
                nc.sync.dma_start(o_ori_r[:, g0:g1, :], os_)

            # ---- preamble, ordered so the first n-tile's dependencies
            # (feat[0], ident, L1 weights) land before the rest of the
            # 4.4MB weight traffic
            load_xtm(0)
            ident = conv(ident_d, [P, P], F32, "ident")
            w, bcol = {}, {}
            w["b", 0] = conv(wd["b", 0].rearrange("(ko p) h -> p ko h", p=P),
                             [P, KO, H], F32R, "w0b")
            bcol["b", 0] = conv(bd["b", 0].rearrange("(m p) -> p m", p=P),
                                [P, KO], F32, "b0b")
            transpose_in(0)
            for head, li in (("b", 1), ("c", 0), ("c", 1)):
                w[head, li] = conv(
                    wd[head, li].rearrange("(ko p) h -> p ko h", p=P),
                    [P, KO, H], F32R, f"w{li}{head}")
                bcol[head, li] = conv(
                    bd[head, li].rearrange("(m p) -> p m", p=P),
                    [P, KO], F32, f"b{li}{head}")
            w["b", 2] = conv(wd["b", 2].rearrange("(ko p) h -> p ko h", p=P),
                             [P, KO, 4], F32R, "w2b")
            w["c", 2] = conv(wd["c", 2].rearrange("(ko p) h -> p ko h", p=P),
                             [P, KO, C], F32R, "w2c")
            b3col = {
                "b": conv(bd["b", 2][:, None], [4, 1], F32, "b3b"),
                "c": conv(bd["c", 2][:, None], [C, 1], F32, "b3c"),
            }
            tri_i = conv(tri_i_d, [T, T], F32, "tri_i")

            # ---- main loop
            for nt in range(NT):
                if nt + 1 < NT:
                    load_xtm(nt + 1)
                    transpose_in(nt + 1)
                h1b = mlp_layer(xT[nt], w["b", 0], bcol["b", 0], "h1b")
                h1c = mlp_layer(xT[nt], w["c", 0], bcol["c", 0], "h1c")
                h2b = mlp_layer(h1b, w["b", 1], bcol["b", 1], "h2b")
                h2c = mlp_layer(h1c, w["c", 1], bcol["c", 1], "h2c")
                fmb = fm_stage(nt, h2b, "b")
                fmc = fm_stage(nt, h2c, "c")
                tb_stage(nt, fmb, "b")
                tb_stage(nt, fmc, "c")
                emit_outputs(nt)
                x_tm[nt] = xT[nt] = None

            # ---- track IDs (closed form, [t, q] layout after PE transpose)
            msk = [idpool.tile([P, T], F32, tag=f"msk{h}", name=f"msk{h}")
                   for h in range(2)]
            for h in range(2):
                nc.vector.tensor_scalar(msk[h][:], ml[h][:], 0.0, None, ALU.is_ge)
            m24 = idpool.tile([T, Q], F32, tag="m24")
            for h in range(2):
                pt = psB.tile([T, P], F32, tag="fm", name="psS")
                nc.tensor.transpose(pt[:], msk[h][:], ident[:])
                nc.vector.tensor_copy(m24[:, h * P:(h + 1) * P], pt[:])
            c24p = psB.tile([T, Q], F32, tag="fm", name="psC")
            nc.tensor.matmul(c24p[:], tri_i[:], m24[:], start=True, stop=True)
            vmask = idpool.tile([T, Q], F32, tag="vmask")
            nc.vector.tensor_scalar(vmask[:], c24p[:], 0.5, None, ALU.is_ge)
            born = idpool.tile([T, Q], F32, tag="born")
            nc.vector.tensor_scalar(born[:], c24p[:], 1.0, None, ALU.is_equal)
            nc.vector.tensor_tensor(born[:], born[:], m24[:], ALU.mult)
            # excl[t] = #tracks born before frame t = sum_q [C - M >= 1]
            vprev = idpool.tile([T, Q], F32, tag="vprev")
            nc.vector.tensor_tensor(vprev[:], c24p[:], m24[:], ALU.subtract)
            nc.vector.tensor_scalar(vprev[:], vprev[:], 0.5, None, ALU.is_ge)
            excl = idpool.tile([T, 1], F32, tag="excl")
            nc.vector.tensor_reduce(excl[:], vprev[:], axis=AX.X, op=ALU.add)
            # inclusive prefix over q in one DVE scan; at born positions the
            # inclusive sum is qprefix_excl+1, so pick = born*(incl + excl)
            # directly encodes ID+1.
            zeros = idpool.tile([T, Q], F32, tag="zeros")
            nc.vector.memset(zeros[:], 0)
            incl = idpool.tile([T, Q], F32, tag="incl")
            nc.vector.tensor_tensor_scan(incl[:], born[:], zeros[:], 0.0,
                                         ALU.add, ALU.add)
            nc.vector.tensor_scalar(incl[:], incl[:], excl[:], None, ALU.add)
            pick = idpool.tile([T, Q], F32, tag="pick")
            nc.vector.tensor_tensor(pick[:], born[:], incl[:], ALU.mult)
            # cumsum over t of the one-hot picks -> (ID+1) from t0 onward
            idsp = psB.tile([T, Q], F32, tag="fm", name="psC2")
            nc.tensor.matmul(idsp[:], tri_i[:], pick[:], start=True, stop=True)
            ids_f = idpool.tile([T, Q], F32, tag="ids_f")
            nc.vector.tensor_tensor(ids_f[:], vmask[:], idsp[:], ALU.mult)
            nc.vector.tensor_scalar(ids_f[:], ids_f[:], 1.0, None, ALU.subtract)
            ids_i = idpool.tile([T, Q], I32, tag="ids_i")
            nc.vector.tensor_copy(ids_i[:], ids_f[:])
            nc.sync.dma_start(o_ids, ids_i[:])

    _split_multiwait(nc)
    return nc


_NC = None


def kernel(object_features,
           bbox_w0, bbox_b0, bbox_w1, bbox_b1, bbox_w2, bbox_b2,
           cls_w0, cls_b0, cls_w1, cls_b1, cls_w2, cls_b2):
    global _NC, LAST_EXEC_TIME_NS
    if _NC is None:
        _NC = _build()
    nc = _NC

    consts = {
        "ident": np.eye(P, dtype=np.float32),
        "tri_incl": np.triu(np.ones((T, T), np.float32)),
    }
    shared = {
        "w0b": bbox_w0, "w1b": bbox_w1, "w2b": bbox_w2,
        "b0b": bbox_b0, "b1b": bbox_b1, "b2b": bbox_b2,
        "w0c": cls_w0, "w1c": cls_w1, "w2c": cls_w2,
        "b0c": cls_b0, "b1c": cls_b1, "b2c": cls_b2,
    }
    shared = {k: np.ascontiguousarray(np.asarray(v, np.float32)) for k, v in shared.items()}
    feats = np.ascontiguousarray(np.asarray(object_features, np.float32))

    in_maps = []
    for c in range(B):
        m = {"feat": feats[c].reshape(NTOK, D)}
        m.update(shared)
        m.update(consts)
        in_maps.append(m)

    r = bass_utils.run_bass_kernel_spmd(nc, in_maps, core_ids=list(range(B)),
                                        trace=TRACE)
    LAST_EXEC_TIME_NS = r.exec_time_ns

    bbox_x = np.stack([r.results[c]["o_bbox"].reshape(T, Q, 4) for c in range(B)])
    cls_x = np.stack([r.results[c]["o_cls"].reshape(T, Q, C) for c in range(B)])
    ori = np.stack([r.results[c]["o_ori"].reshape(T, Q, 4) for c in range(B)])
    ids = np.stack([r.results[c]["o_ids"] for c in range(B)]).astype(np.int32)
    return bbox_x, cls_x, ori, ids


# revision 32
# speedup vs baseline: 1.6649x; 1.0025x over previous
"""BBox+Cls decoder TRN2 kernel (data-parallel over batch, 8 NeuronCores).

Per core: one batch element -> 6144 tokens x 512 features through two
3-layer MLP heads (bbox->4, cls->91), sigmoid, cxcywh->xyxy transform and a
closed-form track-ID assignment (the reference's sequential scan reduces to
cumulative sums because IDs, once assigned, never change).

Matmuls run as float32r (1 PE cycle/row at N>=256 vs 4 for fp32; ~1e-4 rel
error measured on HW). Activations stay feature-major ([D, tokens]) so each
layer feeds the next without transposes; the input and the small final-layer
outputs are transposed on the PE. Outputs are sigmoided and DMA'd per
512-token tile so the stores hide under compute.
"""

import numpy as np

import concourse.bass as bass
import concourse.mybir as mybir
import concourse.tile as tile
from concourse import bass_utils

F32 = mybir.dt.float32
F32R = mybir.dt.float32r
I32 = mybir.dt.int32
ALU = mybir.AluOpType
ACTF = mybir.ActivationFunctionType
AX = mybir.AxisListType

B, T, Q, D, H, C = 8, 24, 256, 512, 512, 91
IMG_W, IMG_H = 1088.0, 608.0
P = 128
NTOK = T * Q            # 6144 tokens per core
NTILE = 512             # tokens per n-tile
NT = NTOK // NTILE      # 12 n-tiles
GPT = NTILE // P        # 4 token groups (of 128) per n-tile
G = NTOK // P           # 48 token groups per core
KO = D // P             # 4 contraction chunks

TRACE = False
LAST_EXEC_TIME_NS = None


def _split_multiwait(nc):
    # This walrus build rejects >1 sync-wait per engine instruction; hoist
    # extras onto preceding same-engine NoOps (the sequencer executes waits
    # in program order, so semantics are identical).
    for f in nc.m.functions:
        for blk in f.blocks:
            out, changed = [], False
            for ins in blk.instructions:
                si = ins.sync_info
                if si is not None and len(si.on_wait) > 1:
                    waits = list(si.on_wait)
                    for j, w in enumerate(waits[:-1]):
                        nd = mybir.InstNoOp(name=f"{ins.name}_wsplit{j}", ins=[], outs=[])
                        nd.engine = ins.engine
                        nd.sync_info = mybir.SyncInfo(on_wait=[w], on_update=[])
                        out.append(nd)
                    si.on_wait = waits[-1:]
                    ins.sync_info = si
                    changed = True
                out.append(ins)
            if changed:
                blk.instructions = out


def _build():
    nc = bass.Bass("TRN2", target_bir_lowering=False, debug=False, num_devices=8)

    feat = nc.dram_tensor("feat", [NTOK, D], F32, kind="ExternalInput").ap()
    wd, bd = {}, {}
    for head, fin in (("b", 4), ("c", C)):
        for li, (n, k) in enumerate([(D, H), (H, H), (H, fin)]):
            wd[head, li] = nc.dram_tensor(f"w{li}{head}", [n, k], F32, kind="ExternalInput").ap()
            bd[head, li] = nc.dram_tensor(f"b{li}{head}", [k], F32, kind="ExternalInput").ap()
    ident_d = nc.dram_tensor("ident", [P, P], F32, kind="ExternalInput").ap()
    tri_i_d = nc.dram_tensor("tri_incl", [T, T], F32, kind="ExternalInput").ap()

    o_bbox = nc.dram_tensor("o_bbox", [NTOK, 4], F32, kind="ExternalOutput").ap()
    o_cls = nc.dram_tensor("o_cls", [NTOK, C], F32, kind="ExternalOutput").ap()
    o_ori = nc.dram_tensor("o_ori", [NTOK, 4], F32, kind="ExternalOutput").ap()
    o_ids = nc.dram_tensor("o_ids", [T, Q], I32, kind="ExternalOutput").ap()

    o_bbox_r = o_bbox.rearrange("(g p) c -> p g c", p=P)
    o_cls_r = o_cls.rearrange("(g p) c -> p g c", p=P)
    o_ori_r = o_ori.rearrange("(g p) c -> p g c", p=P)

    with tile.TileContext(nc) as tc:
        with (
            tc.tile_pool(name="const", bufs=1) as cpool,
            tc.tile_pool(name="io", bufs=2) as iopool,
            tc.tile_pool(name="act", bufs=2) as apool,
            tc.tile_pool(name="big", bufs=1) as bigpool,
            tc.tile_pool(name="idp", bufs=1) as idpool,
            tc.tile_pool(name="psT", bufs=2, space="PSUM") as psT,
            tc.tile_pool(name="psH", bufs=4, space="PSUM") as psH,
            tc.tile_pool(name="psB", bufs=2, space="PSUM") as psB,
        ):
            # ---- constants: DMA fp32 then DVE-convert (to f32r where used
            # by f32r matmuls; consumers then sync on the DVE semaphore
            # instead of scattered DMA lanes).
            def conv(dram_ap, shape, dt, name):
                raw = cpool.tile(shape, F32, tag=f"{name}_raw", name=f"{name}_raw")
                nc.sync.dma_start(raw[:], dram_ap)
                t = cpool.tile(shape, dt, tag=name, name=name)
                nc.vector.tensor_copy(t[:], raw[:])
                return t

            # ---- persistent output staging
            clsbuf = bigpool.tile([P, G, C], F32)        # cls logits -> sigmoid in place
            bb_all = bigpool.tile([P, G, 4], F32)        # bbox logits -> sigmoid in place
            ori_all = bigpool.tile([P, G, 4], F32)
            ml = [bigpool.tile([P, T], F32, tag=f"ml{h}", name=f"ml{h}")
                  for h in range(2)]

            feat_r = feat.rearrange("(nt g p) d -> nt p g d", nt=NT, p=P)

            x_tm = [None] * NT
            xT = [None] * NT

            def load_xtm(nt):
                t = iopool.tile([P, GPT, D], F32, tag="x_tm", name="x_tm")
                nc.sync.dma_start(t[:], feat_r[nt])
                x_tm[nt] = t

            def transpose_in(nt):
                # x_tm [tok(P), g, d] -> xT [d(P), ko, tok] via PE transpose
                xt = apool.tile([P, KO, NTILE], F32R, tag="xT", name="xT")
                for g in range(GPT):
                    pt = psT.tile([P, KO * P], F32, name="psTt")
                    for ko in range(KO):
                        nc.tensor.transpose(
                            pt[:, ko * P:(ko + 1) * P],
                            x_tm[nt][:, g, ko * P:(ko + 1) * P],
                            ident[:],
                        )
                    nc.scalar.copy(
                        xt[:, :, g * P:(g + 1) * P],
                        pt[:].rearrange("p (ko q) -> p ko q", ko=KO),
                    )
                xT[nt] = xt

            def mlp_layer(rhs, wt, bias, out_tag):
                # rhs [P, KO, NTILE] f32r; out h^T [P, KO(m), NTILE] f32r
                out = apool.tile([P, KO, NTILE], F32R, tag=out_tag, name=out_tag)
                for m in range(KO):
                    pt = psH.tile([P, NTILE], F32, name="psHt")
                    for ko in range(KO):
                        nc.tensor.matmul(
                            pt[:], wt[:, ko, m * P:(m + 1) * P], rhs[:, ko, :],
                            start=(ko == 0), stop=(ko == KO - 1))
                    if m < 2:
                        nc.vector.tensor_scalar(
                            out[:, m, :], pt[:], bias[:, m:m + 1], 0.0,
                            ALU.add, ALU.max)
                    else:
                        nc.scalar.activation(
                            out[:, m, :], pt[:], ACTF.Relu,
                            bias=bias[:, m:m + 1], scale=1.0)
                return out

            def fm_stage(nt, h2, head):
                # feature-major final: fm [fin, ntile] = W3^T @ h2  (f32r,
                # N=512 full speed), bias folded into the PSUM->SBUF copy
                fin = 4 if head == "b" else C
                fm_ps = psB.tile([fin, NTILE], F32, tag="fm", name="fm_ps")
                for ko in range(KO):
                    nc.tensor.matmul(fm_ps[:], w[head, 2][:, ko, :], h2[:, ko, :],
                                     start=(ko == 0), stop=(ko == KO - 1))
                fm = apool.tile([fin, NTILE], F32, tag=f"fm{head}", name=f"fm{head}")
                nc.scalar.add(fm[:], fm_ps[:], b3col[head][:])
                return fm

            def tb_stage(nt, fm, head):
                # PE transpose-back to token-major 128-token tiles
                fin = 4 if head == "b" else C
                for g in range(GPT):
                    j = nt * GPT + g
                    tb = psT.tile([P, fin], F32, tag="psTt", name="tb_ps")
                    nc.tensor.transpose(tb[:], fm[:, g * P:(g + 1) * P],
                                        ident[:fin, :fin])
                    if head == "b":
                        nc.vector.tensor_copy(bb_all[:, j, :], tb[:])
                    else:
                        nc.vector.tensor_copy(clsbuf[:, j, :], tb[:])
                        nc.vector.tensor_reduce(
                            ml[j % 2][:, j // 2:j // 2 + 1], clsbuf[:, j, :],
                            axis=AX.X, op=ALU.max)

            uw = idpool.tile([P, GPT], F32, tag="uw")
            uh = idpool.tile([P, GPT], F32, tag="uh")

            def emit_outputs(nt):
                g0, g1 = nt * GPT, (nt + 1) * GPT
                # cls: sigmoid in place, stream out
                cs = clsbuf[:, g0:g1, :]
                nc.scalar.activation(cs.rearrange("p g c -> p (g c)"),
                                     cs.rearrange("p g c -> p (g c)"), ACTF.Sigmoid)
                nc.sync.dma_start(o_cls_r[:, g0:g1, :], cs)
                # bbox: sigmoid in place, compute ori, stream both
                bs = bb_all[:, g0:g1, :]
                nc.scalar.activation(bs.rearrange("p g c -> p (g c)"),
                                     bs.rearrange("p g c -> p (g c)"), ACTF.Sigmoid)
                nc.vector.tensor_scalar(uw[:], bs[:, :, 2], 0.5, None, ALU.mult)
                nc.vector.tensor_scalar(uh[:], bs[:, :, 3], 0.5, None, ALU.mult)
                os_ = ori_all[:, g0:g1, :]
                for col, ci, u, s in ((0, 0, uw, IMG_W), (1, 1, uh, IMG_H),
                                      (2, 0, uw, IMG_W), (3, 1, uh, IMG_H)):
                    op = ALU.subtract if col < 2 else ALU.add
                    nc.vector.tensor_tensor(os_[:, :, col], bs[:, :, ci], u[:], op)
                    nc.vector.tensor_scalar(os_[:, :, col], os_[:, :, col],
                                            s, None, ALU.mult)
                nc.sync.dma_start(o_bbox_r[:, g0:g1, :], bs)
                nc.sync.dma_start(o_ori_r[:, g0:g1, :], os_)

            # ---- preamble, ordered so the first n-tile's dependencies
            # (feat[0], ident, L1 weights) land before the rest of the
            # 4.4MB weight traffic
            load_xtm(0)
            ident = conv(ident_d, [P, P], F32, "ident")
            w, bcol = {}, {}
            w["b", 0] = conv(wd["b", 0].rearrange("(ko p) h -> p ko h", p=P),
                             [P, KO, H], F32R, "w0b")
            bcol["b", 0] = conv(bd["b", 0].rearrange("(m p) -> p m", p=P),
                                [P, KO], F32, "b0b")
            transpose_in(0)
            for head, li in (("b", 1), ("c", 0), ("c", 1)):
                w[head, li] = conv(
                    wd[head, li].rearrange("(ko p) h -> p ko h", p=P),
                    [P, KO, H], F32R, f"w{li}{head}")
                bcol[head, li] = conv(
                    bd[head, li].rearrange("(m p) -> p m", p=P),
                    [P, KO], F32, f"b{li}{head}")
            w["b", 2] = conv(wd["b", 2].rearrange("(ko p) h -> p ko h", p=P),
                             [P, KO, 4], F32R, "w2b")
            w["c", 2] = conv(wd["c", 2].rearrange("(ko p) h -> p ko h", p=P),
                             [P, KO, C], F32R, "w2c")
            b3col = {
                "b": conv(bd["b", 2][:, None], [4, 1], F32, "b3b"),
                "c": conv(bd["c", 2][:, None], [C, 1], F32, "b3c"),
            }
            tri_i = conv(tri_i_d, [T, T], F32, "tri_i")

            # ---- main loop
            for nt in range(NT):
                if nt + 1 < NT:
                    load_xtm(nt + 1)
                    transpose_in(nt + 1)
                h1b = mlp_layer(xT[nt], w["b", 0], bcol["b", 0], "h1b")
                h1c = mlp_layer(xT[nt], w["c", 0], bcol["c", 0], "h1c")
                h2b = mlp_layer(h1b, w["b", 1], bcol["b", 1], "h2b")
                h2c = mlp_layer(h1c, w["c", 1], bcol["c", 1], "h2c")
                fmb = fm_stage(nt, h2b, "b")
                fmc = fm_stage(nt, h2c, "c")
                tb_stage(nt, fmb, "b")
                tb_stage(nt, fmc, "c")
                emit_outputs(nt)
                x_tm[nt] = xT[nt] = None

            # ---- track IDs (closed form, [t, q] layout after PE transpose)
            msk = [idpool.tile([P, T], F32, tag=f"msk{h}", name=f"msk{h}")
                   for h in range(2)]
            for h in range(2):
                nc.vector.tensor_scalar(msk[h][:], ml[h][:], 0.0, None, ALU.is_ge)
            m24 = idpool.tile([T, Q], F32, tag="m24")
            for h in range(2):
                pt = psB.tile([T, P], F32, tag="fm", name="psS")
                nc.tensor.transpose(pt[:], msk[h][:], ident[:])
                nc.vector.tensor_copy(m24[:, h * P:(h + 1) * P], pt[:])
            c24p = psB.tile([T, Q], F32, tag="fm", name="psC")
            nc.tensor.matmul(c24p[:], tri_i[:], m24[:], start=True, stop=True)
            vmask = idpool.tile([T, Q], F32, tag="vmask")
            nc.vector.tensor_scalar(vmask[:], c24p[:], 0.5, None, ALU.is_ge)
            born = idpool.tile([T, Q], F32, tag="born")
            nc.vector.tensor_scalar(born[:], c24p[:], 1.0, None, ALU.is_equal)
            nc.vector.tensor_tensor(born[:], born[:], m24[:], ALU.mult)
            # excl[t] = #tracks born before frame t = sum_q [C - M >= 1]
            vprev = idpool.tile([T, Q], F32, tag="vprev")
            nc.vector.tensor_tensor(vprev[:], c24p[:], m24[:], ALU.subtract)
            nc.vector.tensor_scalar(vprev[:], vprev[:], 0.5, None, ALU.is_ge)
            excl = idpool.tile([T, 1], F32, tag="excl")
            nc.vector.tensor_reduce(excl[:], vprev[:], axis=AX.X, op=ALU.add)
            # inclusive prefix over q in one DVE scan; at born positions the
            # inclusive sum is qprefix_excl+1, so pick = born*(incl + excl)
            # directly encodes ID+1.
            zeros = idpool.tile([T, Q], F32, tag="zeros")
            nc.vector.memset(zeros[:], 0)
            incl = idpool.tile([T, Q], F32, tag="incl")
            nc.vector.tensor_tensor_scan(incl[:], born[:], zeros[:], 0.0,
                                         ALU.add, ALU.add)
            nc.vector.tensor_scalar(incl[:], incl[:], excl[:], None, ALU.add)
            pick = idpool.tile([T, Q], F32, tag="pick")
            nc.vector.tensor_tensor(pick[:], born[:], incl[:], ALU.mult)
            # cumsum over t of the one-hot picks -> (ID+1) from t0 onward
            idsp = psB.tile([T, Q], F32, tag="fm", name="psC2")
            nc.tensor.matmul(idsp[:], tri_i[:], pick[:], start=True, stop=True)
            ids_f = idpool.tile([T, Q], F32, tag="ids_f")
            nc.vector.tensor_tensor(ids_f[:], vmask[:], idsp[:], ALU.mult)
            nc.vector.tensor_scalar(ids_f[:], ids_f[:], 1.0, None, ALU.subtract)
            ids_i = idpool.tile([T, Q], I32, tag="ids_i")
            nc.vector.tensor_copy(ids_i[:], ids_f[:])
            nc.sync.dma_start(o_ids, ids_i[:])

    _split_multiwait(nc)
    return nc


_NC = None


def kernel(object_features,
           bbox_w0, bbox_b0, bbox_w1, bbox_b1, bbox_w2, bbox_b2,
           cls_w0, cls_b0, cls_w1, cls_b1, cls_w2, cls_b2):
    global _NC, LAST_EXEC_TIME_NS
    if _NC is None:
        _NC = _build()
    nc = _NC

    consts = {
        "ident": np.eye(P, dtype=np.float32),
        "tri_incl": np.triu(np.ones((T, T), np.float32)),
    }
    shared = {
        "w0b": bbox_w0, "w1b": bbox_w1, "w2b": bbox_w2,
        "b0b": bbox_b0, "b1b": bbox_b1, "b2b": bbox_b2,
        "w0c": cls_w0, "w1c": cls_w1, "w2c": cls_w2,
        "b0c": cls_b0, "b1c": cls_b1, "b2c": cls_b2,
    }
    shared = {k: np.ascontiguousarray(np.asarray(v, np.float32)) for k, v in shared.items()}
    feats = np.ascontiguousarray(np.asarray(object_features, np.float32))

    in_maps = []
    for c in range(B):
        m = {"feat": feats[c].reshape(NTOK, D)}
        m.update(shared)
        m.update(consts)
        in_maps.append(m)

    r = bass_utils.run_bass_kernel_spmd(nc, in_maps, core_ids=list(range(B)),
                                        trace=TRACE)
    LAST_EXEC_TIME_NS = r.exec_time_ns

    bbox_x = np.stack([r.results[c]["o_bbox"].reshape(T, Q, 4) for c in range(B)])
    cls_x = np.stack([r.results[c]["o_cls"].reshape(T, Q, C) for c in range(B)])
    ori = np.stack([r.results[c]["o_ori"].reshape(T, Q, 4) for c in range(B)])
    ids = np.stack([r.results[c]["o_ids"] for c in range(B)]).astype(np.int32)
    return bbox_x, cls_x, ori, ids
